# revision 1
# baseline (speedup 1.0000x reference)
"""Trainium2 Bass kernel for nn_Cross_MultiViewAtt (B=16, C=512, 32x32, 8 heads).

Self-contained: kernel(**inputs) -> (16, 512, 32, 32) float32.

Strategy (8 NeuronCores, SPMD, two launches):
  Launch 1 runs the three transformer blocks (h, v, s) sharded over each
  attention's independent batch axis (W=32 -> 4 cols/core for h and v;
  1024 block tokens -> 128/core for s), activations feature-major
  (channels on SBUF partitions), bf16 activation stream with fp32 PSUM
  accumulation. Host resharding, then launch 2 runs conv1 +
  cross-attention + conv2 sharded over batch (2 images/core).

  Launch-1 fast path (used when all biases are zero and LN affines are
  trivial, which holds for this module's init): LN1 reduces to mean
  subtraction only -- the 1/std scale commutes through the bias-free
  ReLU FF and cancels in LN2's per-token normalization. rsqrt is
  computed as exp(-0.5*ln(var+eps)) so every activation function used
  lives in one hardware table (no act-table reloads).
"""
import os
import sys

for _p in ('/root/.axon_site/_ro/trn_rl_repo', '/opt/trn_rl_repo'):
    if os.path.isdir(_p) and _p not in sys.path:
        sys.path.append(_p)

import numpy as np
import ml_dtypes

import concourse.bass as bass
import concourse.tile as tile
from concourse import bacc, mybir
from concourse.alu_op_type import AluOpType as OP
from concourse.bass import ts
F32 = mybir.dt.float32
F32R = mybir.dt.float32r
BF16 = mybir.dt.bfloat16
AF = mybir.ActivationFunctionType
BF = ml_dtypes.bfloat16

N_CORES = 8
C = 512
NH = 8
HD = 64          # head dim
B = 16
HW = 32
T = 2048         # tokens per core per block
NCH = 4          # 512-token chunks per block
XSHIFT = 60.0    # cross-attention exp shift (scores range +-82, row max >= 31)
EPS = 1e-5


def r32(ap):
    return ap.bitcast(F32R)


# ---------------------------------------------------------------------------
# Engine-dispatch helpers: 'a' = Activation, 'd' = DVE, 'p' = Pool/GpSimd
# ---------------------------------------------------------------------------

def ev_copy(nc, eng, dst, src, bias=None):
    """dst = src (+ per-partition bias ptr), psum/sbuf eviction."""
    if eng == 'a':
        nc.scalar.activation(dst, src, AF.Identity,
                             bias=bias if bias is not None else 0.0)
    elif eng == 'd':
        if bias is None:
            nc.vector.tensor_copy(dst, src)
        else:
            nc.vector.tensor_scalar(dst, src, bias, None, OP.add)
    else:
        if bias is None:
            nc.gpsimd.tensor_copy(dst, src)
        else:
            nc.gpsimd.tensor_scalar(dst, src, bias, None, OP.add)


def ev_relu(nc, eng, dst, src, bias=None):
    """dst = relu(src + bias)."""
    if eng == 'a':
        nc.scalar.activation(dst, src, AF.Relu,
                             bias=bias if bias is not None else 0.0)
    elif eng == 'd':
        nc.vector.tensor_scalar(dst, src, bias if bias is not None else 0.0,
                                0.0, OP.add, OP.max)
    else:
        nc.gpsimd.tensor_scalar(dst, src, bias if bias is not None else 0.0,
                                0.0, OP.add, OP.max)


def ev_res(nc, eng, dst, ps, res, bias=None):
    """dst = ps + res (+ per-partition bias ptr): residual eviction."""
    e = nc.vector if eng == 'd' else (nc.gpsimd if eng == 'p' else None)
    if e is not None:
        if bias is None:
            e.tensor_tensor(dst, ps, res, OP.add)
        else:
            e.scalar_tensor_tensor(dst, ps, bias, res, OP.add, OP.add)
    else:
        raise ValueError("residual eviction only on d/p")


def ev_mult(nc, eng, dst, a, b):
    e = {'d': nc.vector, 'p': nc.gpsimd}[eng]
    e.tensor_tensor(dst, a, b, OP.mult)


# ---------------------------------------------------------------------------
# Launch 1 device program
# ---------------------------------------------------------------------------

BLOCKS = [
    dict(name='h', L=512, NLOC=4, FFD=1024),
    dict(name='v', L=512, NLOC=4, FFD=512),
    dict(name='s', L=16, NLOC=128, FFD=1024),
]

# engine assignment per site, per block kind ('hv' or 's').
# Pool/GpSimd cannot access PSUM, so PSUM evictions only go to 'a'/'d';
# Pool takes SBUF-to-SBUF work (broadcasts are hardcoded to Pool).
ASSIGN = {
    'hv': dict(
        qk=['d'] * 8,
        v=['d'] * 4,
        onorm=['d'] * 8,
        ff1=['a', 'a', 'a', 'a', 'd', 'd', 'd', 'd'],
        lnu=['d', 'd', 'd', 'd'],
        tsq=['p', 'p', 'd', 'd'],
        z=['p', 'p', 'p', 'p', 'd', 'd', 'd', 'd'],
        em=['d'] * 8,
    ),
    's': dict(
        qk=['a', 'a', 'a', 'd', 'a', 'a', 'a', 'd'],
        v=['a', 'd', 'a', 'd'],
        onorm=['d'] * 8,
        ff1=['a', 'a', 'a', 'a', 'd', 'd', 'd', 'd'],
        lnu=['d', 'd', 'd', 'd'],
        tsq=['p', 'p', 'd', 'd'],
        z=['p', 'p', 'p', 'p', 'd', 'd', 'd', 'd'],
        em=['p'] * 8,
    ),
}


def build_launch1(trivial):
    nc = bacc.Bacc("TRN2", target_bir_lowering=False, debug=False, num_devices=N_CORES)
    ins = {}
    outs = {}
    for cfg in BLOCKS:
        p = cfg['name']
        ffd = cfg['FFD']
        ins[f'x{p}'] = nc.dram_tensor(f'x{p}', [C, T], BF16, kind="ExternalInput")
        ins[f'{p}_wqk'] = nc.dram_tensor(f'{p}_wqk', [C, 2 * C], BF16, kind="ExternalInput")
        ins[f'{p}_wv'] = nc.dram_tensor(f'{p}_wv', [C, C], BF16, kind="ExternalInput")
        ins[f'{p}_wo'] = nc.dram_tensor(f'{p}_wo', [C, C], BF16, kind="ExternalInput")
        ins[f'{p}_w1'] = nc.dram_tensor(f'{p}_w1', [C, ffd], BF16, kind="ExternalInput")
        ins[f'{p}_w2'] = nc.dram_tensor(f'{p}_w2', [ffd, C], BF16, kind="ExternalInput")
        if not trivial:
            ins[f'{p}_bqk'] = nc.dram_tensor(f'{p}_bqk', [2 * C, 1], F32, kind="ExternalInput")
            ins[f'{p}_bv'] = nc.dram_tensor(f'{p}_bv', [C, 1], BF16, kind="ExternalInput")
            ins[f'{p}_bo'] = nc.dram_tensor(f'{p}_bo', [C, 1], F32, kind="ExternalInput")
            ins[f'{p}_b1'] = nc.dram_tensor(f'{p}_b1', [ffd, 1], F32, kind="ExternalInput")
            ins[f'{p}_b2'] = nc.dram_tensor(f'{p}_b2', [C, 1], F32, kind="ExternalInput")
            ins[f'{p}_ln1g'] = nc.dram_tensor(f'{p}_ln1g', [C, 1], F32, kind="ExternalInput")
            ins[f'{p}_ln1b'] = nc.dram_tensor(f'{p}_ln1b', [C, 1], F32, kind="ExternalInput")
            ins[f'{p}_ln2g'] = nc.dram_tensor(f'{p}_ln2g', [C, 1], F32, kind="ExternalInput")
            ins[f'{p}_ln2b'] = nc.dram_tensor(f'{p}_ln2b', [C, 1], F32, kind="ExternalInput")
        outs[f'y{p}'] = nc.dram_tensor(f'y{p}', [C, T], F32, kind="ExternalOutput")
    ins['smask'] = nc.dram_tensor('smask', [128, 512], BF16, kind="ExternalInput")
    ins['onesv'] = nc.dram_tensor('onesv', [128, 1], BF16, kind="ExternalInput")

    with tile.TileContext(nc) as tc:
        from contextlib import ExitStack
        with ExitStack() as octx:
            octx.enter_context(nc.allow_low_precision(reason="bf16 matmul pipeline"))
            const = octx.enter_context(tc.tile_pool(name="const", bufs=1))
            ones_bf = const.tile([128, 1], BF16, tag="ones_bf", name="ones_bf")
            nc.sync.dma_start(ones_bf[:], ins['onesv'].ap()[:, 0:1])
            epst = const.tile([1, 1], F32, tag="epst", name="epst")
            nc.vector.memset(epst[:], EPS)
            smask = const.tile([128, 512], BF16, tag="smask", name="smask")
            nc.sync.dma_start(smask[:], ins['smask'].ap()[:, :])

            wp = octx.enter_context(tc.tile_pool(name="blk_w", bufs=1))
            pers = octx.enter_context(tc.tile_pool(name="blk_pers", bufs=1))
            psum = octx.enter_context(tc.tile_pool(name="blk_ps", bufs=1, space="PSUM"))
            strm = octx.enter_context(tc.tile_pool(name="blk_strm", bufs=1))
            tmp = octx.enter_context(tc.tile_pool(name="blk_tmp", bufs=1))
            bcp = octx.enter_context(tc.tile_pool(name="blk_bc", bufs=1))
            pools = (wp, pers, psum, strm, tmp, bcp)
            for cfg in BLOCKS:
                _tblock_dev(tc, octx, nc, cfg, ins, outs,
                            ones_bf, epst, smask, pools, trivial)
    nc.compile()
    return nc


def _tblock_dev(tc, octx, nc, cfg, ins, outs, ones_bf, epst, smask, pools, trivial):
    p = cfg['name']
    ffd = cfg['FFD']
    FJ = ffd // 128
    wp, pers, psum, strm, tmp, bcp = pools
    asg = ASSIGN['s' if p == 's' else 'hv']

    # ---- load x and weights (x + wqk first: first qk matmul needs them) ----
    x = [pers.tile([128, T], BF16, tag=f"x{i}", name=f"x{i}") for i in range(4)]
    for i in range(4):
        nc.sync.dma_start(x[i][:, ts(0, 512)],
                          ins[f'x{p}'].ap()[ts(i, 128), ts(0, 512)])
    wqk = [wp.tile([128, 1024], BF16, tag=f"{p}wqk{i}", name=f"wqk{i}") for i in range(4)]
    wv = [wp.tile([128, 512], BF16, tag=f"{p}wv{i}", name=f"wv{i}") for i in range(4)]
    wo = [wp.tile([128, 512], BF16, tag=f"{p}wo{i}", name=f"wo{i}") for i in range(4)]
    w1 = [wp.tile([128, ffd], BF16, tag=f"{p}w1_{i}", name=f"w1_{i}") for i in range(4)]
    w2 = [wp.tile([128, 512], BF16, tag=f"{p}w2_{i}", name=f"w2_{i}") for i in range(FJ)]
    for i in range(4):
        nc.sync.dma_start(wqk[i][:], ins[f'{p}_wqk'].ap()[ts(i, 128), :])
    for ch0 in range(1, NCH):
        for i in range(4):
            nc.sync.dma_start(x[i][:, ts(ch0, 512)],
                              ins[f'x{p}'].ap()[ts(i, 128), ts(ch0, 512)])
    for i in range(4):
        nc.sync.dma_start(wv[i][:], ins[f'{p}_wv'].ap()[ts(i, 128), :])
        nc.sync.dma_start(wo[i][:], ins[f'{p}_wo'].ap()[ts(i, 128), :])
        nc.sync.dma_start(w1[i][:], ins[f'{p}_w1'].ap()[ts(i, 128), :])
    for i in range(FJ):
        nc.sync.dma_start(w2[i][:], ins[f'{p}_w2'].ap()[ts(i, 128), :])

    if not trivial:
        bqk = wp.tile([128, 8], F32, tag="bqk", name="bqk")
        nc.sync.dma_start(bqk[:], ins[f'{p}_bqk'].ap().rearrange("(j q) one -> q (j one)", q=128))
        bo = wp.tile([128, 4], F32, tag="bo", name="bo")
        nc.sync.dma_start(bo[:], ins[f'{p}_bo'].ap().rearrange("(j q) one -> q (j one)", q=128))
        b1 = wp.tile([128, FJ], F32, tag="b1", name="b1")
        nc.sync.dma_start(b1[:], ins[f'{p}_b1'].ap().rearrange("(j q) one -> q (j one)", q=128))
        b2 = wp.tile([128, 4], F32, tag="b2", name="b2")
        nc.sync.dma_start(b2[:], ins[f'{p}_b2'].ap().rearrange("(j q) one -> q (j one)", q=128))
        ln1g = wp.tile([128, 4], F32, tag="ln1g", name="ln1g")
        nc.sync.dma_start(ln1g[:], ins[f'{p}_ln1g'].ap().rearrange("(j q) one -> q (j one)", q=128))
        ln1b = wp.tile([128, 4], F32, tag="ln1b", name="ln1b")
        nc.sync.dma_start(ln1b[:], ins[f'{p}_ln1b'].ap().rearrange("(j q) one -> q (j one)", q=128))
        ln2g = wp.tile([128, 4], F32, tag="ln2g", name="ln2g")
        nc.sync.dma_start(ln2g[:], ins[f'{p}_ln2g'].ap().rearrange("(j q) one -> q (j one)", q=128))
        ln2b = wp.tile([128, 4], F32, tag="ln2b", name="ln2b")
        nc.sync.dma_start(ln2b[:], ins[f'{p}_ln2b'].ap().rearrange("(j q) one -> q (j one)", q=128))
        # v-bias as a broadcast tile
        bvrow = wp.tile([1, 512], BF16, tag="bvrow", name="bvrow")
        nc.sync.dma_start(bvrow[:], ins[f'{p}_bv'].ap().rearrange("c one -> one c"))
        bv_bc = wp.tile([128, 512], BF16, tag="bvbc", name="bvbc")
        nc.gpsimd.partition_broadcast(bv_bc[:], bvrow[:])
    else:
        bqk = bo = b1 = b2 = ln1g = ln1b = ln2g = ln2b = bv_bc = None

    # ================= per 512-token chunk =====================
    # QKV for chunk ch+1 is emitted as 12 thunks interleaved into the
    # attention head loop of chunk ch, so the PE never starves while the
    # Activation engine cranks through the softmax exps.
    def emit_qkv_plan(ch):
        cols = ts(ch, 512)
        qksb = [strm.tile([128, 512], BF16, tag=f"qk{j}", bufs=2, name=f"qk{j}")
                for j in range(8)]
        # v tiles are (128, 8*65): per head 64 value cols + a ones column so
        # the AV matmul's stationary [v_h | 1] emits the softmax row-sum as
        # psum row 64 for free.
        vt = [strm.tile([128, 520], BF16, tag=f"v{ti}", bufs=2, name=f"v{ti}")
              for ti in range(4)]

        def qk_thunk(j):
            ps = psum.tile([128, 512], F32, tag="acc", bufs=5, name="qk_ps")
            for ci in range(4):
                nc.tensor.matmul(ps[:], wqk[ci][:, ts(j, 128)], x[ci][:, cols],
                                 start=(ci == 0), stop=(ci == 3))
            ev_copy(nc, asg['qk'][j], qksb[j][:], ps[:],
                    None if trivial else bqk[:, j:j + 1])

        def v_thunk(ti):
            ps = psum.tile([128, 512], F32, tag="acc", bufs=5, name="v_ps")
            for ci in range(4):
                nc.tensor.matmul(ps[:], x[ci][:, ch * 512 + ti * 128:ch * 512 + (ti + 1) * 128],
                                 wv[ci][:], start=(ci == 0), stop=(ci == 3))
            sb = vt[ti]
            dst = sb[:].rearrange("k (h e) -> k h e", e=65)[:, :, 0:64]
            src = ps[:].rearrange("k (h e) -> k h e", e=64)
            if trivial:
                ev_copy(nc, asg['v'][ti], dst, src)
            else:
                ev_res(nc, 'd', dst, src,
                       bv_bc[:].rearrange("k (h e) -> k h e", e=64))
            nc.vector.memset(sb[:].rearrange("k (h e) -> k h e", e=65)[:, :, 64:65], 1.0)

        thunks = [lambda j=j: qk_thunk(j) for j in range(8)]
        thunks += [lambda ti=ti: v_thunk(ti) for ti in range(4)]
        qh = [qksb[hb // 2][(hb % 2) * 64:(hb % 2) * 64 + 64, :] for hb in range(8)]
        kh = [qksb[4 + hb // 2][(hb % 2) * 64:(hb % 2) * 64 + 64, :] for hb in range(8)]
        return thunks, qh, kh, vt

    thunks, qh, kh, vt = emit_qkv_plan(0)
    for t in thunks:
        t()
    for ch in range(NCH):
        cols = ts(ch, 512)
        if ch + 1 < NCH:
            nthunks, nqh, nkh, nvt = emit_qkv_plan(ch + 1)
        else:
            nthunks = []
        ndone = [0]

        def drain_to(want, _nt=nthunks, _nd=ndone):
            while _nd[0] < min(want, len(_nt)):
                _nt[_nd[0]]()
                _nd[0] += 1

        def interleave(h, _d=drain_to):
            _d(((h + 1) * 11) // 8)

        # ---- attention -> o_fm tiles (4 x (128,512) bf16) ----
        o_fm = [strm.tile([128, 512], BF16, tag=f"o{j}", bufs=1, name=f"o{j}") for j in range(4)]
        if p in ('h', 'v'):
            _attn_hv(nc, psum, strm, tmp, bcp, qh, kh, vt, o_fm, ones_bf, asg,
                     interleave)
        else:
            _attn_s(nc, psum, strm, tmp, bcp, qh, kh, vt, o_fm, ones_bf, smask,
                    asg, interleave)
        # hold the last thunk back: it fills the PE bubble while the LN1
        # mean row is reduced/broadcast (emitted below)
        drain_to(11)
        if ch + 1 < NCH:
            qh, kh, vt = nqh, nkh, nvt

        # ---- out projection + residual (+ LN1 mean handling) ----
        for j in range(4):
            ps = psum.tile([128, 512], F32, tag="acc", bufs=5, name="y_ps")
            for ci in range(4):
                nc.tensor.matmul(ps[:], wo[ci][:, ts(j, 128)], o_fm[ci][:],
                                 start=(ci == 0), stop=(ci == 3))
            ev_res(nc, 'd', x[j][:, cols], ps[:], x[j][:, cols],
                   None if trivial else bo[:, j:j + 1])
        if trivial:
            # LN1 fast path: subtract the per-token mean only; the 1/std
            # scale commutes through the bias-free ReLU FF and cancels in
            # LN2 (positive per-token scale invariance).
            srow = psum.tile([33, 512], F32, tag="row", bufs=1, name="ln1_s1")
            for j in range(4):
                nc.tensor.matmul(srow[0:1, :], ones_bf[:], x[j][:, cols],
                                 start=(j == 0), stop=(j == 3))
            nm = tmp.tile([1, 512], BF16, tag="ln1_nm", bufs=1, name="ln1_nm")
            nc.vector.tensor_scalar(nm[:], srow[0:1, :], -1.0 / C, None, OP.mult)
            nm_bc = bcp.tile([128, 512], BF16, tag="ln1_nmbc", bufs=2, name="ln1_nmbc")
            nc.gpsimd.partition_broadcast(nm_bc[:], nm[:])
            drain_to(12)
            for j in range(4):
                ev_res(nc, asg['lnu'][j], x[j][:, cols], x[j][:, cols], nm_bc[:])
        else:
            _layernorm_full(nc, psum, tmp, bcp, x, cols, ln1g, ln1b, epst,
                            ones_bf, lambda j: (x[j][:, cols], None), tmp, asg)
            drain_to(12)

        # ---- ff1: relu(t @ w1 + b1) -> hb tiles ----
        hb = []
        for j in range(FJ):
            ps = psum.tile([128, 512], F32, tag="acc", bufs=5, name="f1_ps")
            for ci in range(4):
                nc.tensor.matmul(ps[:], w1[ci][:, ts(j, 128)], x[ci][:, cols],
                                 start=(ci == 0), stop=(ci == 3))
            sb = strm.tile([128, 512], BF16, tag=f"hb{j}", bufs=1, name=f"hb{j}")
            ev_relu(nc, asg['ff1'][j % len(asg['ff1'])], sb[:], ps[:],
                    None if trivial else b1[:, j:j + 1])
            hb.append(sb)
        # ---- ff2 + residual (overwrites x) ; LN2 -> dram ----
        for j in range(4):
            ps = psum.tile([128, 512], F32, tag="acc", bufs=5, name="f2_ps")
            for ci in range(FJ):
                nc.tensor.matmul(ps[:], w2[ci][:, ts(j, 128)], hb[ci][:],
                                 start=(ci == 0), stop=(ci == FJ - 1))
            ev_res(nc, 'd', x[j][:, cols], ps[:], x[j][:, cols],
                   None if trivial else b2[:, j:j + 1])

        def _dest(j, _ch=ch):
            zt = tmp.tile([128, 512], F32, tag="ln_out", bufs=2, name=f"lnout{j}")
            return (zt[:], lambda: nc.sync.dma_start(
                outs[f'y{p}'].ap()[ts(j, 128), ts(_ch, 512)], zt[:]))
        _layernorm_full(nc, psum, tmp, bcp, x, cols, ln2g, ln2b, epst,
                        ones_bf, _dest, tmp, asg)


def _layernorm_full(nc, psum, tmp, bcp, x, cols, g_t, b_t, epst, ones_bf,
                    dest_fn, tmpp, asg):
    """Full LayerNorm over C (partition dim) for one 512-token chunk.

    x: 4 bf16 tiles; operates on columns `cols`. A = rsqrt(var+eps) is
    computed as exp(-0.5*ln(var+eps)) to stay inside one act table.
    g_t/b_t None => trivial affine.
    """
    srow = psum.tile([33, 512], F32, tag="row", bufs=1, name="ln_s12")
    for j in range(4):
        nc.tensor.matmul(srow[0:1, :], ones_bf[:], x[j][:, cols],
                         start=(j == 0), stop=(j == 3))
    for j in range(4):
        tsq = tmpp.tile([128, 512], BF16, tag="ln_tsq", bufs=2, name="ln_tsq")
        ev_mult(nc, asg['tsq'][j], tsq[:], x[j][:, cols], x[j][:, cols])
        nc.tensor.matmul(srow[32:33, :], ones_bf[:], tsq[:],
                         start=(j == 0), stop=(j == 3))
    m = tmp.tile([1, 512], F32, tag="ln_m", bufs=1, name="ln_m")
    nc.vector.tensor_scalar(m[:], srow[0:1, :], 1.0 / C, None, OP.mult)
    mm = tmp.tile([1, 512], F32, tag="ln_mm", bufs=1, name="ln_mm")
    nc.vector.tensor_tensor(mm[:], m[:], m[:], OP.mult)
    var = tmp.tile([1, 512], F32, tag="ln_var", bufs=1, name="ln_var")
    nc.vector.scalar_tensor_tensor(var[:], srow[32:33, :], 1.0 / C, mm[:], OP.mult, OP.subtract)
    lnv = tmp.tile([1, 512], F32, tag="ln_lnv", bufs=1, name="ln_lnv")
    nc.scalar.activation(lnv[:], var[:], AF.Ln, bias=epst[:1, :])
    A = tmp.tile([1, 512], F32, tag="ln_A", bufs=1, name="ln_A")
    nc.scalar.activation(A[:], lnv[:], AF.Exp, scale=-0.5)
    Bv = tmp.tile([1, 512], F32, tag="ln_B", bufs=1, name="ln_B")
    nc.vector.scalar_tensor_tensor(Bv[:], m[:], -1.0, A[:], OP.mult, OP.mult)
    A_bc = bcp.tile([128, 512], F32, tag="ln_Abc", bufs=2, name="ln_Abc")
    nc.gpsimd.partition_broadcast(A_bc[:], A[:])
    B_bc = bcp.tile([128, 512], F32, tag="ln_Bbc", bufs=2, name="ln_Bbc")
    nc.gpsimd.partition_broadcast(B_bc[:], Bv[:])
    for j in range(4):
        u = tmp.tile([128, 512], F32, tag="ln_u", bufs=2, name="ln_u")
        ev_mult(nc, asg['z'][2 * j], u[:], x[j][:, cols], A_bc[:])
        dest, post = dest_fn(j)
        if g_t is None:
            ev_res(nc, asg['z'][2 * j + 1], dest, u[:], B_bc[:])
        else:
            z = tmp.tile([128, 512], F32, tag="ln_z", bufs=2, name="ln_z")
            nc.vector.tensor_tensor(z[:], u[:], B_bc[:], OP.add)
            nc.scalar.activation(dest, z[:], AF.Identity,
                                 bias=b_t[:, j:j + 1], scale=g_t[:, j:j + 1])
        if post is not None:
            post()


def _attn_hv(nc, psum, strm, tmp, bcp, qh, kh, vt, o_fm, ones_bf, asg, interleave):
    """Attention for h/v blocks: one batch n per 512-chunk, L=512, 8 heads.

    The AV stationary is [v_h | 1] (65 wide), so o_ps row 64 holds the
    softmax row-sum -- no separate row-sum matmuls.
    """
    for h in range(8):
        off = (h % 2) * 64
        e_t = []
        for mt in range(4):
            sps = psum.tile([128, 512], F32, tag="acc", bufs=5, name="s_ps")
            nc.tensor.matmul(sps[:], kh[h][:, ts(mt, 128)], qh[h][:],
                             start=True, stop=True)
            e = strm.tile([128, 512], BF16, tag="e", bufs=5, name="e")
            nc.scalar.activation(e[:], sps[:], AF.Exp, scale=0.125)
            e_t.append(e)
        o_ps = psum.tile([65, 512], F32, tag="ops", bufs=2, name="o_ps")
        for mt in range(4):
            nc.tensor.matmul(o_ps[:], vt[mt][:, h * 65:h * 65 + 65],
                             e_t[mt][:], start=(mt == 0), stop=(mt == 3))
        rinv = tmp.tile([1, 512], F32, tag="rinv", bufs=4, name="rinv")
        nc.vector.reciprocal(rinv[:], o_ps[64:65, :])
        R_bc = bcp.tile([64, 512], F32, tag="rbc", bufs=4, name="rbc")
        nc.gpsimd.partition_broadcast(R_bc[:], rinv[:])
        ev_mult(nc, asg['onorm'][h], o_fm[h // 2][off:off + 64, :],
                o_ps[0:64, :], R_bc[:])
        interleave(h)


def _attn_s(nc, psum, strm, tmp, bcp, qh, kh, vt, o_fm, ones_bf, smask, asg, interleave):
    """Attention for the s block: 512-chunk = 32 batches of L=16; masked dense."""
    for h in range(8):
        off = (h % 2) * 64
        sps = psum.tile([128, 512], F32, tag="acc", bufs=5, name="ss_ps")
        for g in range(4):
            nc.tensor.matmul(sps[:, ts(g, 128)], kh[h][:, ts(g, 128)],
                             qh[h][:, ts(g, 128)], start=True, stop=True)
        e = strm.tile([128, 512], BF16, tag="es", bufs=3, name="es")
        nc.scalar.activation(e[:], sps[:], AF.Exp, scale=0.125)
        em = strm.tile([128, 512], BF16, tag="em", bufs=3, name="em")
        ev_mult(nc, asg['em'][h], em[:], e[:], smask[:])
        o_ps = psum.tile([65, 512], F32, tag="ops", bufs=2, name="os_ps")
        for g in range(4):
            nc.tensor.matmul(o_ps[:, ts(g, 128)], vt[g][:, h * 65:h * 65 + 65],
                             em[:, ts(g, 128)], start=True, stop=True)
        rinv = tmp.tile([1, 512], F32, tag="rinv", bufs=4, name="rinvs")
        nc.vector.reciprocal(rinv[:], o_ps[64:65, :])
        R_bc = bcp.tile([64, 512], F32, tag="rbc", bufs=4, name="rbcs")
        nc.gpsimd.partition_broadcast(R_bc[:], rinv[:])
        ev_mult(nc, asg['onorm'][h], o_fm[h // 2][off:off + 64, :],
                o_ps[0:64, :], R_bc[:])
        interleave(h)


# ---------------------------------------------------------------------------
# Launch 2 device program
# ---------------------------------------------------------------------------

def build_launch2():
    nc = bacc.Bacc("TRN2", target_bir_lowering=False, debug=False, num_devices=N_CORES)
    NPIX = 1024
    ins = {}
    for bb in range(2):
        ins[f'hc{bb}'] = nc.dram_tensor(f'hc{bb}', [C, NPIX], F32R, kind="ExternalInput")
        ins[f'vc{bb}'] = nc.dram_tensor(f'vc{bb}', [C, NPIX], F32R, kind="ExternalInput")
        ins[f'q4_{bb}'] = nc.dram_tensor(f'q4_{bb}', [C, NPIX], F32R, kind="ExternalInput")
    ins['c1w'] = nc.dram_tensor('c1w', [2 * C, C], F32R, kind="ExternalInput")
    ins['c2w'] = nc.dram_tensor('c2w', [C, C], F32R, kind="ExternalInput")
    ins['c1b'] = nc.dram_tensor('c1b', [C, 1], F32, kind="ExternalInput")
    ins['c2b'] = nc.dram_tensor('c2b', [C, 1], F32, kind="ExternalInput")
    ins['onesv'] = nc.dram_tensor('onesv', [128, 2], F32R, kind="ExternalInput")
    ins['ident'] = nc.dram_tensor('ident', [128, 128], F32R, kind="ExternalInput")
    outs = {}
    for bb in range(2):
        outs[f'out{bb}'] = nc.dram_tensor(f'out{bb}', [C, NPIX], F32, kind="ExternalOutput")

    with tile.TileContext(nc) as tc:
        from contextlib import ExitStack
        with ExitStack() as ctx:
            ctx.enter_context(nc.allow_low_precision(reason="fp32r matmul input pipeline"))
            const = ctx.enter_context(tc.tile_pool(name="const", bufs=1))
            ones128 = const.tile([128, 2], F32R, tag="ones128", name="ones128")
            nc.sync.dma_start(ones128[:], ins['onesv'].ap()[:, :])
            onesrow = const.tile([1, 128], F32R, tag="onesrow", name="onesrow")
            nc.sync.dma_start(onesrow[:], ins['onesv'].ap()[:, 0:1].rearrange("c one -> one c"))
            nshift = const.tile([128, 1], F32, tag="nshift", name="nshift")
            nc.vector.memset(nshift[:], -XSHIFT)
            ident = const.tile([128, 128], F32R, tag="ident", name="ident")
            nc.sync.dma_start(ident[:], ins['ident'].ap()[:, :])
            wp = ctx.enter_context(tc.tile_pool(name="w", bufs=1))
            c1w = [wp.tile([128, 512], F32R, tag=f"c1w{i}", name=f"c1w{i}") for i in range(8)]
            for i in range(8):
                nc.sync.dma_start(c1w[i][:], ins['c1w'].ap()[ts(i, 128), :])
            c2w = [wp.tile([128, 512], F32R, tag=f"c2w{i}", name=f"c2w{i}") for i in range(4)]
            for i in range(4):
                nc.sync.dma_start(c2w[i][:], ins['c2w'].ap()[ts(i, 128), :])
            c1b = wp.tile([128, 4], F32, tag="c1b", name="c1b")
            nc.sync.dma_start(c1b[:], ins['c1b'].ap().rearrange("(j q) one -> q (j one)", q=128))
            c2b = wp.tile([128, 4], F32, tag="c2b", name="c2b")
            nc.sync.dma_start(c2b[:], ins['c2b'].ap().rearrange("(j q) one -> q (j one)", q=128))
            c1brow = wp.tile([1, 512], F32R, tag="c1brow", name="c1brow")
            nc.sync.dma_start(c1brow[:], ins['c1b'].ap().rearrange("c one -> one c").bitcast(F32R))
            psum0 = ctx.enter_context(tc.tile_pool(name="ps0", bufs=1, space="PSUM"))
            c1b_bc = wp.tile([128, 512], F32, tag="c1bbc", name="c1bbc")
            nc.gpsimd.partition_broadcast(c1b_bc[:], c1brow[:].bitcast(F32))

            dramp = ctx.enter_context(tc.tile_pool(name="dram", bufs=2, space="DRAM"))

            pers2 = ctx.enter_context(tc.tile_pool(name="b_pers", bufs=1))
            psum2 = ctx.enter_context(tc.tile_pool(name="b_ps", bufs=1, space="PSUM"))
            strm2 = ctx.enter_context(tc.tile_pool(name="b_strm", bufs=1))
            for bb in range(2):
                _launch2_b(tc, ctx, nc, bb, ins, outs, c1w, c2w, c1b, c2b, c1b_bc,
                           ones128, nshift, dramp, pers2, psum2, strm2, ident)
    nc.compile()
    return nc


def _launch2_b(tc, octx, nc, bb, ins, outs, c1w, c2w, c1b, c2b, c1b_bc,
               ones128, nshift, dramp, pers, psum, strm, ident):
    if True:
        hc = [pers.tile([128, 1024], F32R, tag=f"hc{i}", name=f"hc{i}") for i in range(4)]
        vc = [pers.tile([128, 1024], F32R, tag=f"vc{i}", name=f"vc{i}") for i in range(4)]
        q4 = [pers.tile([128, 1024], F32R, tag=f"q4{i}", name=f"q4{i}") for i in range(4)]
        for i in range(4):
            nc.sync.dma_start(hc[i][:], ins[f'hc{bb}'].ap()[ts(i, 128), :])
            nc.sync.dma_start(vc[i][:], ins[f'vc{bb}'].ap()[ts(i, 128), :])
            nc.sync.dma_start(q4[i][:], ins[f'q4_{bb}'].ap()[ts(i, 128), :])
        fused = hc + vc

        # conv1 feature-major: gm_fm (512, 1024)
        gm_fm = [pers.tile([128, 1024], F32R, tag=f"gmf{j}", name=f"gmf{j}") for j in range(4)]
        for pj in range(2):
            for j in range(4):
                ps = psum.tile([128, 512], F32, tag="acc", bufs=5, name="g1_ps")
                for ci in range(8):
                    nc.tensor.matmul(ps[:], r32(c1w[ci][:, ts(j, 128)]),
                                     r32(fused[ci][:, ts(pj, 512)]),
                                     start=(ci == 0), stop=(ci == 7))
                nc.scalar.activation(gm_fm[j][:, ts(pj, 512)], ps[:], AF.Identity,
                                     bias=c1b[:, j:j + 1])
        # conv1 token-major via PE transpose of gm_fm (bias already applied)
        gm_tok = [pers.tile([128, 512], F32R, tag=f"gmt{t}", name=f"gmt{t}") for t in range(8)]
        for t in range(8):
            ps = psum.tile([128, 512], F32, tag="acc", bufs=5, name="g2_ps")
            for ci in range(4):
                nc.tensor.transpose(r32(ps[:, ts(ci, 128)]), r32(gm_fm[ci][:, ts(t, 128)]),
                                    ident[:])
            if t % 2 == 0:
                nc.vector.tensor_copy(gm_tok[t][:], ps[:])
            else:
                nc.scalar.activation(gm_tok[t][:], ps[:], AF.Identity)

        # scores + exp: e (1024 m, 1024 t)
        e_t = [pers.tile([128, 1024], F32R, tag=f"e{mt}", name=f"e{mt}") for mt in range(8)]
        for mt in range(8):
            for tj in range(2):
                ps = psum.tile([128, 512], F32, tag="acc", bufs=5, name="sc_ps")
                for ci in range(4):
                    nc.tensor.matmul(ps[:], r32(gm_fm[ci][:, ts(mt, 128)]),
                                     r32(q4[ci][:, ts(tj, 512)]),
                                     start=(ci == 0), stop=(ci == 3))
                nc.scalar.activation(e_t[mt][:, ts(tj, 512)], ps[:], AF.Exp,
                                     bias=nshift[:, :])
        # att (token-major) + row sums; normalize via per-partition recip
        att_dram = dramp.tile([1024, 512], F32, tag="attd", name="attd")
        for tt in range(8):
            aps = psum.tile([128, 512], F32, tag="acc", bufs=5, name="at_ps")
            rps = psum.tile([128, 2], F32, tag="row", bufs=1, name="r_ps")
            for mt in range(8):
                nc.tensor.matmul(aps[:], r32(e_t[mt][:, ts(tt, 128)]), r32(gm_tok[mt][:]),
                                 start=(mt == 0), stop=(mt == 7))
            for mt in range(8):
                nc.tensor.matmul(rps[:], r32(e_t[mt][:, ts(tt, 128)]), r32(ones128[:]),
                                 start=(mt == 0), stop=(mt == 7))
            rinv = strm.tile([128, 1], F32, tag="rinv", bufs=1, name="rinv")
            nc.vector.reciprocal(rinv[:], rps[:, 0:1])
            att = strm.tile([128, 512], F32, tag="att", bufs=3, name="att")
            nc.vector.tensor_scalar(att[:], aps[:], rinv[:], None, OP.mult)
            nc.sync.dma_start(att_dram[ts(tt, 128), :], att[:])

        # scrambled view: S_i = flat(att_i) as (512, 256); z = gm + S; conv2
        z_t = [pers.tile([128, 1024], F32R, tag=f"z{j}", name=f"z{j}") for j in range(4)]
        att_flat = att_dram[:].rearrange("n c -> (n c)")
        for i in range(4):
            y0 = (i // 2) * 16
            x0 = (i % 2) * 16
            for j in range(4):
                S = strm.tile([128, 256], F32, tag="S", bufs=3, name="S")
                src = att_flat[i * 131072 + j * 32768: i * 131072 + (j + 1) * 32768]
                nc.sync.dma_start(S[:], src.rearrange("(q f) -> q f", q=128))
                gm_view = gm_fm[j][:].rearrange("q (h w) -> q h w", h=32)[
                    :, y0:y0 + 16, x0:x0 + 16]
                nc.vector.tensor_tensor(z_t[j][:, i * 256:(i + 1) * 256].rearrange("q (y x) -> q y x", y=16),
                                        S[:].rearrange("q (y x) -> q y x", y=16),
                                        gm_view, OP.add)
        # conv2: out = c2w @ z + c2b
        for pj in range(2):
            for j in range(4):
                ps = psum.tile([128, 512], F32, tag="acc", bufs=5, name="o_ps")
                for ci in range(4):
                    nc.tensor.matmul(ps[:], r32(c2w[ci][:, ts(j, 128)]),
                                     r32(z_t[ci][:, ts(pj, 512)]),
                                     start=(ci == 0), stop=(ci == 3))
                ot = strm.tile([128, 512], F32, tag="ot", bufs=3, name="ot")
                nc.scalar.activation(ot[:], ps[:], AF.Identity, bias=c2b[:, j:j + 1])
                nc.sync.dma_start(outs[f'out{bb}'].ap()[ts(j, 128), ts(pj, 512)], ot[:])


# ---------------------------------------------------------------------------
# Host-side sharding / resharding
# ---------------------------------------------------------------------------

def make_smask():
    m = np.zeros((128, 128), np.float32)
    for n in range(8):
        m[n * 16:(n + 1) * 16, n * 16:(n + 1) * 16] = 1.0
    return np.tile(m, (1, 4)).copy()


def _is_trivial(inputs):
    for p in ('h', 'v', 's'):
        for k, want in ((f'{p}_qkv_b', 0.0), (f'{p}_out_b', 0.0),
                        (f'{p}_ff1_b', 0.0), (f'{p}_ff2_b', 0.0),
                        (f'{p}_ln1_g', 1.0), (f'{p}_ln1_b', 0.0),
                        (f'{p}_ln2_g', 1.0), (f'{p}_ln2_b', 0.0)):
            if not np.all(np.asarray(inputs[k]) == want):
                return False
    return True


def block_weights(inputs, p, trivial):
    wqkv = np.asarray(inputs[f'{p}_qkv_w'], np.float32)
    d = {
        f'{p}_wqk': np.ascontiguousarray(wqkv[:1024].T).astype(BF),
        f'{p}_wv': np.ascontiguousarray(wqkv[1024:1536].T).astype(BF),
        f'{p}_wo': np.ascontiguousarray(np.asarray(inputs[f'{p}_out_w'], np.float32).T).astype(BF),
        f'{p}_w1': np.ascontiguousarray(np.asarray(inputs[f'{p}_ff1_w'], np.float32).T).astype(BF),
        f'{p}_w2': np.ascontiguousarray(np.asarray(inputs[f'{p}_ff2_w'], np.float32).T).astype(BF),
    }
    if not trivial:
        d[f'{p}_bqk'] = np.asarray(inputs[f'{p}_qkv_b'][:1024], np.float32).reshape(-1, 1)
        d[f'{p}_bv'] = np.asarray(inputs[f'{p}_qkv_b'][1024:1536], np.float32).reshape(-1, 1).astype(BF)
        d[f'{p}_bo'] = np.asarray(inputs[f'{p}_out_b'], np.float32).reshape(-1, 1)
        d[f'{p}_b1'] = np.asarray(inputs[f'{p}_ff1_b'], np.float32).reshape(-1, 1)
        d[f'{p}_b2'] = np.asarray(inputs[f'{p}_ff2_b'], np.float32).reshape(-1, 1)
        d[f'{p}_ln1g'] = np.asarray(inputs[f'{p}_ln1_g'], np.float32).reshape(-1, 1)
        d[f'{p}_ln1b'] = np.asarray(inputs[f'{p}_ln1_b'], np.float32).reshape(-1, 1)
        d[f'{p}_ln2g'] = np.asarray(inputs[f'{p}_ln2_g'], np.float32).reshape(-1, 1)
        d[f'{p}_ln2b'] = np.asarray(inputs[f'{p}_ln2_b'], np.float32).reshape(-1, 1)
    return d


def make_bq(fm):
    b, c, h, w = fm.shape
    y = fm.reshape(b, 32, 16, 2, 16, w)
    y = np.transpose(y, (0, 1, 3, 5, 2, 4))
    blocks = y.reshape(b, c, 2, 2, 16, 16)
    blk = np.transpose(blocks, (0, 2, 3, 1, 4, 5)).reshape(b, 4, c, 256)
    bq = np.transpose(blk, (0, 1, 3, 2)).reshape(b, 4 * 256, c)
    return np.ascontiguousarray(bq)


def shard_launch1(inputs, trivial):
    fm = np.asarray(inputs['feature_map'], np.float32)
    b, c, h, w = fm.shape
    xh_full = np.transpose(fm, (0, 2, 3, 1)).reshape(b * h, w, c)   # (L, W, C)
    xv_full = np.transpose(fm, (0, 3, 2, 1)).reshape(b * w, h, c)   # (L, H, C)
    bq = make_bq(fm)                                                # (B, 1024, C)
    weights = {}
    for p in ('h', 'v', 's'):
        weights.update(block_weights(inputs, p, trivial))
    weights['smask'] = make_smask().astype(BF)
    weights['onesv'] = np.ones((128, 1), BF)
    in_maps = []
    for core in range(N_CORES):
        m = dict(weights)
        xh = xh_full[:, core * 4:(core + 1) * 4, :]          # (512, 4, C)
        m['xh'] = np.ascontiguousarray(np.transpose(xh, (2, 1, 0)).reshape(C, T)).astype(BF)
        xv = xv_full[:, core * 4:(core + 1) * 4, :]
        m['xv'] = np.ascontiguousarray(np.transpose(xv, (2, 1, 0)).reshape(C, T)).astype(BF)
        xs = bq[:, core * 128:(core + 1) * 128, :]           # (16, 128, C)
        m['xs'] = np.ascontiguousarray(np.transpose(xs, (2, 1, 0)).reshape(C, T)).astype(BF)
        in_maps.append(m)
    return in_maps


def reshard_launch2(results1, inputs):
    """results1: list per core of {'yh','yv','ys'} feature-major (C, 2048) bf16."""
    Hc = np.zeros((B, C, HW, HW), np.float32)
    Vc = np.zeros((B, C, HW, HW), np.float32)
    bq2 = np.zeros((B, 1024, C), np.float32)
    for core in range(N_CORES):
        yh = np.asarray(results1[core]['yh'], dtype=np.float32).reshape(C, 4, B, HW)
        Hc[:, :, :, core * 4:(core + 1) * 4] = np.transpose(yh, (2, 0, 3, 1))
        yv = np.asarray(results1[core]['yv'], dtype=np.float32).reshape(C, 4, B, HW)
        Vc[:, :, core * 4:(core + 1) * 4, :] = np.transpose(yv, (2, 0, 1, 3))
        ys = np.asarray(results1[core]['ys'], dtype=np.float32).reshape(C, 128, B)
        bq2[:, core * 128:(core + 1) * 128, :] = np.transpose(ys, (2, 1, 0))
    c1w = np.ascontiguousarray(np.asarray(inputs['conv1_w'], np.float32).T)  # (1024, 512)
    c2w = np.ascontiguousarray(np.asarray(inputs['conv2_w'], np.float32).T)  # (512, 512)
    c1b = np.asarray(inputs['conv1_b'], np.float32).reshape(-1, 1)
    c2b = np.asarray(inputs['conv2_b'], np.float32).reshape(-1, 1)
    in_maps = []
    for core in range(N_CORES):
        m = {'c1w': c1w, 'c2w': c2w, 'c1b': c1b, 'c2b': c2b,
             'onesv': np.ones((128, 2), np.float32),
             'ident': np.eye(128, dtype=np.float32)}
        for bb in range(2):
            b_idx = core * 2 + bb
            m[f'hc{bb}'] = np.ascontiguousarray(Hc[b_idx].reshape(C, 1024))
            m[f'vc{bb}'] = np.ascontiguousarray(Vc[b_idx].reshape(C, 1024))
            m[f'q4_{bb}'] = np.ascontiguousarray(bq2[b_idx].T)
        in_maps.append(m)
    return in_maps, (Hc, Vc, bq2)


def unshard_output(results2):
    out = np.zeros((B, C, HW, HW), np.float32)
    for core in range(N_CORES):
        for bb in range(2):
            b_idx = core * 2 + bb
            ob = results2[core][f'out{bb}']                   # (C, 1024) beta-order
            ob = ob.reshape(C, 2, 2, 16, 16)
            out[b_idx] = np.transpose(ob, (0, 1, 3, 2, 4)).reshape(C, HW, HW)
    return out


# ---------------------------------------------------------------------------
# Entry point
# ---------------------------------------------------------------------------

_CACHE = {}


def _programs(trivial=True):
    key = ('nc1', trivial)
    if key not in _CACHE:
        _CACHE[key] = build_launch1(trivial)
    if 'nc2' not in _CACHE:
        _CACHE['nc2'] = build_launch2()
    return _CACHE[key], _CACHE['nc2']


def kernel(**inputs) -> np.ndarray:
    from concourse import bass_utils
    trivial = _is_trivial(inputs)
    nc1, nc2 = _programs(trivial)
    in_maps1 = shard_launch1(inputs, trivial)
    r1 = bass_utils.run_bass_kernel_spmd(nc1, in_maps1, core_ids=list(range(N_CORES)))
    in_maps2, _ = reshard_launch2(r1.results, inputs)
    r2 = bass_utils.run_bass_kernel_spmd(nc2, in_maps2, core_ids=list(range(N_CORES)))
    return unshard_output(r2.results)



# revision 9
# speedup vs baseline: 1.3143x; 1.3143x over previous
"""Trainium2 Bass kernel for nn_Cross_MultiViewAtt (B=16, C=512, 32x32, 8 heads).

Self-contained: kernel(**inputs) -> (16, 512, 32, 32) float32.

Strategy (8 NeuronCores, SPMD, two launches):
  Launch 1 runs the three transformer blocks (h, v, s) sharded over each
  attention's independent batch axis (W=32 -> 4 cols/core for h and v;
  1024 block tokens -> 128/core for s), activations feature-major
  (channels on SBUF partitions), bf16 activation stream with fp32 PSUM
  accumulation. Host resharding, then launch 2 runs conv1 +
  cross-attention + conv2 sharded over batch (2 images/core).

  Launch-1 fast path (used when all biases are zero and LN affines are
  trivial, which holds for this module's init): LN1 reduces to mean
  subtraction only -- the 1/std scale commutes through the bias-free
  ReLU FF and cancels in LN2's per-token normalization. rsqrt is
  computed as exp(-0.5*ln(var+eps)) so every activation function used
  lives in one hardware table (no act-table reloads).
"""
import os
import sys

for _p in ('/root/.axon_site/_ro/trn_rl_repo', '/opt/trn_rl_repo'):
    if os.path.isdir(_p) and _p not in sys.path:
        sys.path.append(_p)

import numpy as np
import ml_dtypes

import concourse.bass as bass
import concourse.tile as tile
from concourse import bacc, mybir
from concourse.alu_op_type import AluOpType as OP
from concourse.bass import ts
F32 = mybir.dt.float32
F32R = mybir.dt.float32r
BF16 = mybir.dt.bfloat16
AF = mybir.ActivationFunctionType
BF = ml_dtypes.bfloat16

N_CORES = 8
C = 512
NH = 8
HD = 64          # head dim
B = 16
HW = 32
T = 2048         # tokens per core per block
NCH = 4          # 512-token chunks per block
XSHIFT = 60.0    # cross-attention exp shift (scores range +-82, row max >= 31)
EPS = 1e-5


def r32(ap):
    return ap.bitcast(F32R)


# ---------------------------------------------------------------------------
# Engine-dispatch helpers: 'a' = Activation, 'd' = DVE, 'p' = Pool/GpSimd
# ---------------------------------------------------------------------------

def ev_copy(nc, eng, dst, src, bias=None):
    """dst = src (+ per-partition bias ptr), psum/sbuf eviction."""
    if eng == 'a':
        nc.scalar.activation(dst, src, AF.Identity,
                             bias=bias if bias is not None else 0.0)
    elif eng == 'd':
        if bias is None:
            nc.vector.tensor_copy(dst, src)
        else:
            nc.vector.tensor_scalar(dst, src, bias, None, OP.add)
    else:
        if bias is None:
            nc.gpsimd.tensor_copy(dst, src)
        else:
            nc.gpsimd.tensor_scalar(dst, src, bias, None, OP.add)


def ev_relu(nc, eng, dst, src, bias=None):
    """dst = relu(src + bias)."""
    if eng == 'a':
        nc.scalar.activation(dst, src, AF.Relu,
                             bias=bias if bias is not None else 0.0)
    elif eng == 'd':
        nc.vector.tensor_scalar(dst, src, bias if bias is not None else 0.0,
                                0.0, OP.add, OP.max)
    else:
        nc.gpsimd.tensor_scalar(dst, src, bias if bias is not None else 0.0,
                                0.0, OP.add, OP.max)


def ev_res(nc, eng, dst, ps, res, bias=None):
    """dst = ps + res (+ per-partition bias ptr): residual eviction."""
    e = nc.vector if eng == 'd' else (nc.gpsimd if eng == 'p' else None)
    if e is not None:
        if bias is None:
            e.tensor_tensor(dst, ps, res, OP.add)
        else:
            e.scalar_tensor_tensor(dst, ps, bias, res, OP.add, OP.add)
    else:
        raise ValueError("residual eviction only on d/p")


def ev_mult(nc, eng, dst, a, b):
    e = {'d': nc.vector, 'p': nc.gpsimd}[eng]
    e.tensor_tensor(dst, a, b, OP.mult)


# ---------------------------------------------------------------------------
# Launch 1 device program
# ---------------------------------------------------------------------------

BLOCKS = [
    dict(name='h', L=512, NLOC=4, FFD=1024),
    dict(name='v', L=512, NLOC=4, FFD=512),
    dict(name='s', L=16, NLOC=128, FFD=1024),
]

# engine assignment per site, per block kind ('hv' or 's').
# Pool/GpSimd cannot access PSUM, so PSUM evictions only go to 'a'/'d';
# Pool takes SBUF-to-SBUF work (broadcasts are hardcoded to Pool).
ASSIGN = {
    'hv': dict(
        qk=['d'] * 8,
        v=['d'] * 4,
        onorm=['d'] * 8,
        ff1=['a', 'a', 'a', 'a', 'd', 'd', 'd', 'd'],
        lnu=['d', 'd', 'd', 'd'],
        tsq=['p', 'p', 'd', 'd'],
        z=['p', 'p', 'p', 'p', 'd', 'd', 'd', 'd'],
        em=['d'] * 8,
    ),
    's': dict(
        qk=['a', 'a', 'a', 'd', 'a', 'a', 'a', 'd'],
        v=['a', 'd', 'a', 'd'],
        onorm=['d'] * 8,
        ff1=['a', 'a', 'a', 'a', 'd', 'd', 'd', 'd'],
        lnu=['d', 'd', 'd', 'd'],
        tsq=['p', 'p', 'd', 'd'],
        z=['p', 'p', 'p', 'p', 'd', 'd', 'd', 'd'],
        em=['p'] * 8,
    ),
}


# ---------------------------------------------------------------------------
# Fast trivial-path launch 1: LN2 deferred to host (launch2 inputs are linear
# in the LN2 output, so the host normalizes exactly during resharding), LN1
# folded into the out-projection (Wo' = Wo - colmean(Wo) on host plus a
# -mean(x) row shipped per block), bf16 outputs, per-block x tiles with all
# input DMAs issued up front so the SP queue never blocks block transitions.
# ---------------------------------------------------------------------------

ASSIGN_FAST = {
    'hv': dict(
        qk=['a', 'd', 'a', 'd', 'a', 'd', 'a', 'd'],
        v=['d', 'd', 'a', 'a'],
        onorm=['d'] * 8,
        ff1=['a', 'd', 'a', 'd', 'a', 'd', 'a', 'd'],
        em=['d'] * 8,
    ),
    's': dict(
        qk=['a'] * 8,
        v=['a', 'd', 'a', 'd'],
        onorm=['d'] * 8,
        ff1=['a', 'd', 'a', 'd', 'a', 'd', 'a', 'd'],
        em=['d'] * 8,
    ),
}


def build_launch1_fast():
    nc = bacc.Bacc("TRN2", target_bir_lowering=False, debug=False, num_devices=N_CORES)
    ins = {}
    outs = {}
    for cfg in BLOCKS:
        p = cfg['name']
        ffd = cfg['FFD']
        ins[f'x{p}'] = nc.dram_tensor(f'x{p}', [C, T], BF16, kind="ExternalInput")
        ins[f'negmx{p}'] = nc.dram_tensor(f'negmx{p}', [1, T], BF16, kind="ExternalInput")
        ins[f'{p}_wqk'] = nc.dram_tensor(f'{p}_wqk', [C, 2 * C], BF16, kind="ExternalInput")
        ins[f'{p}_wv'] = nc.dram_tensor(f'{p}_wv', [C, C], BF16, kind="ExternalInput")
        ins[f'{p}_wo'] = nc.dram_tensor(f'{p}_wo', [C, C], BF16, kind="ExternalInput")
        ins[f'{p}_w1'] = nc.dram_tensor(f'{p}_w1', [C, ffd], BF16, kind="ExternalInput")
        ins[f'{p}_w2'] = nc.dram_tensor(f'{p}_w2', [ffd, C], BF16, kind="ExternalInput")
        outs[f'y{p}'] = nc.dram_tensor(f'y{p}', [C, T], BF16, kind="ExternalOutput")
    ins['smask'] = nc.dram_tensor('smask', [128, 512], BF16, kind="ExternalInput")

    with tile.TileContext(nc) as tc:
        from contextlib import ExitStack
        with ExitStack() as octx:
            octx.enter_context(nc.allow_low_precision(reason="bf16 matmul pipeline"))
            const = octx.enter_context(tc.tile_pool(name="const", bufs=1))
            wp = octx.enter_context(tc.tile_pool(name="blk_w", bufs=1))
            pers = octx.enter_context(tc.tile_pool(name="blk_pers", bufs=1))
            psum = octx.enter_context(tc.tile_pool(name="blk_ps", bufs=1, space="PSUM"))
            strm = octx.enter_context(tc.tile_pool(name="blk_strm", bufs=1))
            tmp = octx.enter_context(tc.tile_pool(name="blk_tmp", bufs=1))
            bcp = octx.enter_context(tc.tile_pool(name="blk_bc", bufs=1))

            # ---- all input DMAs up front (SP queue has no waits here) ----
            xs = {}
            negmx = {}
            wts = {}
            # h's first-chunk x + wqk first so the very first matmul can start
            p0 = BLOCKS[0]['name']
            xs[p0] = [pers.tile([128, T], BF16, tag=f"{p0}x{i}", name=f"x{i}")
                      for i in range(4)]
            for i in range(4):
                nc.sync.dma_start(xs[p0][i][:, ts(0, 512)],
                                  ins[f'x{p0}'].ap()[ts(i, 128), ts(0, 512)])
            for cfg in BLOCKS:
                p = cfg['name']
                ffd = cfg['FFD']
                FJ = ffd // 128
                wqk = [wp.tile([128, 1024], BF16, tag=f"{p}wqk{i}", name=f"wqk{i}")
                       for i in range(4)]
                for i in range(4):
                    nc.sync.dma_start(wqk[i][:], ins[f'{p}_wqk'].ap()[ts(i, 128), :])
                if p != p0:
                    xs[p] = [pers.tile([128, T], BF16, tag=f"{p}x{i}", name=f"x{i}")
                             for i in range(4)]
                    for i in range(4):
                        nc.sync.dma_start(xs[p][i][:], ins[f'x{p}'].ap()[ts(i, 128), :])
                else:
                    for ch0 in range(1, NCH):
                        for i in range(4):
                            nc.sync.dma_start(xs[p][i][:, ts(ch0, 512)],
                                              ins[f'x{p}'].ap()[ts(i, 128), ts(ch0, 512)])
                negmx[p] = ins[f'negmx{p}']
                wv = [wp.tile([128, 512], BF16, tag=f"{p}wv{i}", name=f"wv{i}") for i in range(4)]
                wo = [wp.tile([128, 512], BF16, tag=f"{p}wo{i}", name=f"wo{i}") for i in range(4)]
                w1 = [wp.tile([128, ffd], BF16, tag=f"{p}w1_{i}", name=f"w1_{i}") for i in range(4)]
                w2 = [wp.tile([128, 512], BF16, tag=f"{p}w2_{i}", name=f"w2_{i}") for i in range(FJ)]
                for i in range(4):
                    nc.sync.dma_start(wv[i][:], ins[f'{p}_wv'].ap()[ts(i, 128), :])
                    nc.sync.dma_start(wo[i][:], ins[f'{p}_wo'].ap()[ts(i, 128), :])
                    nc.sync.dma_start(w1[i][:], ins[f'{p}_w1'].ap()[ts(i, 128), :])
                for i in range(FJ):
                    nc.sync.dma_start(w2[i][:], ins[f'{p}_w2'].ap()[ts(i, 128), :])
                wts[p] = (wqk, wv, wo, w1, w2)
            smask = const.tile([128, 512], BF16, tag="smask", name="smask")
            nc.sync.dma_start(smask[:], ins['smask'].ap()[:, :])

            pools = (wp, pers, psum, strm, tmp, bcp)
            for cfg in BLOCKS:
                _tblock_fast(tc, octx, nc, cfg, outs, xs[cfg['name']],
                             negmx[cfg['name']], wts[cfg['name']], smask, pools)
    nc.compile()
    return nc


def _tblock_fast(tc, octx, nc, cfg, outs, x, negmx, wts, smask, pools):
    p = cfg['name']
    ffd = cfg['FFD']
    FJ = ffd // 128
    wp, pers, psum, strm, tmp, bcp = pools
    wqk, wv, wo, w1, w2 = wts
    asg = ASSIGN_FAST['s' if p == 's' else 'hv']

    def emit_qkv_plan(ch):
        cols = ts(ch, 512)
        qksb = [strm.tile([128, 512], BF16, tag=f"qk{j}", bufs=2, name=f"qk{j}")
                for j in range(8)]
        vt = [strm.tile([128, 520], BF16, tag=f"v{ti}", bufs=2, name=f"v{ti}")
              for ti in range(4)]

        def qk_thunk(j):
            ps = psum.tile([128, 512], F32, tag="acc", bufs=6, name="qk_ps")
            for ci in range(4):
                nc.tensor.matmul(ps[:], wqk[ci][:, ts(j, 128)], x[ci][:, cols],
                                 start=(ci == 0), stop=(ci == 3))
            ev_copy(nc, asg['qk'][j], qksb[j][:], ps[:])

        def v_thunk(ti):
            ps = psum.tile([128, 512], F32, tag="acc", bufs=6, name="v_ps")
            for ci in range(4):
                nc.tensor.matmul(ps[:], x[ci][:, ch * 512 + ti * 128:ch * 512 + (ti + 1) * 128],
                                 wv[ci][:], start=(ci == 0), stop=(ci == 3))
            sb = vt[ti]
            dst = sb[:].rearrange("k (h e) -> k h e", e=65)[:, :, 0:64]
            src = ps[:].rearrange("k (h e) -> k h e", e=64)
            ev_copy(nc, asg['v'][ti], dst, src)
            nc.vector.memset(sb[:].rearrange("k (h e) -> k h e", e=65)[:, :, 64:65], 1.0)

        def mx_thunk():
            # xt = x - mean(x) per token: residual source for the folded LN1
            nmrow = tmp.tile([1, 512], BF16, tag="nmrow", bufs=2, name="nmrow")
            nc.scalar.dma_start(nmrow[:], negmx.ap()[:, cols])
            mxbc = bcp.tile([128, 512], BF16, tag="mxbc", bufs=2, name="mxbc")
            nc.gpsimd.partition_broadcast(mxbc[:], nmrow[:])
            for j in range(4):
                xt = strm.tile([128, 512], BF16, tag=f"xt{j}", bufs=2, name=f"xt{j}")
                nc.vector.tensor_tensor(xt[:], x[j][:, cols], mxbc[:], OP.add)
                xts.append(xt)

        xts = []
        thunks = [lambda: mx_thunk()]
        thunks += [lambda j=j: qk_thunk(j) for j in range(8)]
        thunks += [lambda ti=ti: v_thunk(ti) for ti in range(4)]
        qh = [qksb[hb // 2][(hb % 2) * 64:(hb % 2) * 64 + 64, :] for hb in range(8)]
        kh = [qksb[4 + hb // 2][(hb % 2) * 64:(hb % 2) * 64 + 64, :] for hb in range(8)]
        return thunks, qh, kh, vt, xts

    thunks, qh, kh, vt, xts = emit_qkv_plan(0)
    for t in thunks:
        t()
    for ch in range(NCH):
        cols = ts(ch, 512)
        if ch + 1 < NCH:
            nthunks, nqh, nkh, nvt, nxts = emit_qkv_plan(ch + 1)
        else:
            nthunks = []
        ndone = [0]

        def drain_to(want, _nt=nthunks, _nd=ndone):
            while _nd[0] < min(want, len(_nt)):
                _nt[_nd[0]]()
                _nd[0] += 1

        def interleave(h, _d=drain_to):
            _d(((h + 1) * 13) // 8)

        o_fm = [strm.tile([128, 512], BF16, tag=f"o{j}", bufs=1, name=f"o{j}") for j in range(4)]
        if p in ('h', 'v'):
            _attn_hv(nc, psum, strm, tmp, bcp, qh, kh, vt, o_fm, None, asg,
                     interleave, accb=6)
        else:
            _attn_s(nc, psum, strm, tmp, bcp, qh, kh, vt, o_fm, None, smask,
                    asg, interleave, accb=6)
        drain_to(13)

        # ---- out projection (Wo') + xt residual: t = ps + (x - mean(x)) ----
        for j in range(4):
            ps = psum.tile([128, 512], F32, tag="acc", bufs=6, name="y_ps")
            for ci in range(4):
                nc.tensor.matmul(ps[:], wo[ci][:, ts(j, 128)], o_fm[ci][:],
                                 start=(ci == 0), stop=(ci == 3))
            nc.vector.tensor_tensor(x[j][:, cols], ps[:], xts[j][:], OP.add)
        if ch + 1 < NCH:
            qh, kh, vt, xts = nqh, nkh, nvt, nxts

        # ---- ff1: relu(t @ w1) ----
        hb = []
        for j in range(FJ):
            ps = psum.tile([128, 512], F32, tag="acc", bufs=6, name="f1_ps")
            for ci in range(4):
                nc.tensor.matmul(ps[:], w1[ci][:, ts(j, 128)], x[ci][:, cols],
                                 start=(ci == 0), stop=(ci == 3))
            sb = strm.tile([128, 512], BF16, tag=f"hb{j}", bufs=1, name=f"hb{j}")
            ev_relu(nc, asg['ff1'][j % len(asg['ff1'])], sb[:], ps[:])
            hb.append(sb)
        # ---- ff2 + residual -> y (pre-LN2, bf16) -> dram ----
        for j in range(4):
            ps = psum.tile([128, 512], F32, tag="acc", bufs=6, name="f2_ps")
            for ci in range(FJ):
                nc.tensor.matmul(ps[:], w2[ci][:, ts(j, 128)], hb[ci][:],
                                 start=(ci == 0), stop=(ci == FJ - 1))
            nc.vector.tensor_tensor(x[j][:, cols], ps[:], x[j][:, cols], OP.add)
            nc.sync.dma_start(outs[f'y{p}'].ap()[ts(j, 128), ts(ch, 512)],
                              x[j][:, cols])


def build_launch1(trivial):
    if trivial:
        return build_launch1_fast()
    nc = bacc.Bacc("TRN2", target_bir_lowering=False, debug=False, num_devices=N_CORES)
    ins = {}
    outs = {}
    for cfg in BLOCKS:
        p = cfg['name']
        ffd = cfg['FFD']
        ins[f'x{p}'] = nc.dram_tensor(f'x{p}', [C, T], BF16, kind="ExternalInput")
        ins[f'{p}_wqk'] = nc.dram_tensor(f'{p}_wqk', [C, 2 * C], BF16, kind="ExternalInput")
        ins[f'{p}_wv'] = nc.dram_tensor(f'{p}_wv', [C, C], BF16, kind="ExternalInput")
        ins[f'{p}_wo'] = nc.dram_tensor(f'{p}_wo', [C, C], BF16, kind="ExternalInput")
        ins[f'{p}_w1'] = nc.dram_tensor(f'{p}_w1', [C, ffd], BF16, kind="ExternalInput")
        ins[f'{p}_w2'] = nc.dram_tensor(f'{p}_w2', [ffd, C], BF16, kind="ExternalInput")
        if not trivial:
            ins[f'{p}_bqk'] = nc.dram_tensor(f'{p}_bqk', [2 * C, 1], F32, kind="ExternalInput")
            ins[f'{p}_bv'] = nc.dram_tensor(f'{p}_bv', [C, 1], BF16, kind="ExternalInput")
            ins[f'{p}_bo'] = nc.dram_tensor(f'{p}_bo', [C, 1], F32, kind="ExternalInput")
            ins[f'{p}_b1'] = nc.dram_tensor(f'{p}_b1', [ffd, 1], F32, kind="ExternalInput")
            ins[f'{p}_b2'] = nc.dram_tensor(f'{p}_b2', [C, 1], F32, kind="ExternalInput")
            ins[f'{p}_ln1g'] = nc.dram_tensor(f'{p}_ln1g', [C, 1], F32, kind="ExternalInput")
            ins[f'{p}_ln1b'] = nc.dram_tensor(f'{p}_ln1b', [C, 1], F32, kind="ExternalInput")
            ins[f'{p}_ln2g'] = nc.dram_tensor(f'{p}_ln2g', [C, 1], F32, kind="ExternalInput")
            ins[f'{p}_ln2b'] = nc.dram_tensor(f'{p}_ln2b', [C, 1], F32, kind="ExternalInput")
        outs[f'y{p}'] = nc.dram_tensor(f'y{p}', [C, T], F32, kind="ExternalOutput")
    ins['smask'] = nc.dram_tensor('smask', [128, 512], BF16, kind="ExternalInput")
    ins['onesv'] = nc.dram_tensor('onesv', [128, 1], BF16, kind="ExternalInput")

    with tile.TileContext(nc) as tc:
        from contextlib import ExitStack
        with ExitStack() as octx:
            octx.enter_context(nc.allow_low_precision(reason="bf16 matmul pipeline"))
            const = octx.enter_context(tc.tile_pool(name="const", bufs=1))
            ones_bf = const.tile([128, 1], BF16, tag="ones_bf", name="ones_bf")
            nc.sync.dma_start(ones_bf[:], ins['onesv'].ap()[:, 0:1])
            epst = const.tile([1, 1], F32, tag="epst", name="epst")
            nc.vector.memset(epst[:], EPS)
            smask = const.tile([128, 512], BF16, tag="smask", name="smask")
            nc.sync.dma_start(smask[:], ins['smask'].ap()[:, :])

            wp = octx.enter_context(tc.tile_pool(name="blk_w", bufs=1))
            pers = octx.enter_context(tc.tile_pool(name="blk_pers", bufs=1))
            psum = octx.enter_context(tc.tile_pool(name="blk_ps", bufs=1, space="PSUM"))
            strm = octx.enter_context(tc.tile_pool(name="blk_strm", bufs=1))
            tmp = octx.enter_context(tc.tile_pool(name="blk_tmp", bufs=1))
            bcp = octx.enter_context(tc.tile_pool(name="blk_bc", bufs=1))
            pools = (wp, pers, psum, strm, tmp, bcp)
            for cfg in BLOCKS:
                _tblock_dev(tc, octx, nc, cfg, ins, outs,
                            ones_bf, epst, smask, pools, trivial)
    nc.compile()
    return nc


def _tblock_dev(tc, octx, nc, cfg, ins, outs, ones_bf, epst, smask, pools, trivial):
    p = cfg['name']
    ffd = cfg['FFD']
    FJ = ffd // 128
    wp, pers, psum, strm, tmp, bcp = pools
    asg = ASSIGN['s' if p == 's' else 'hv']

    # ---- load x and weights (x + wqk first: first qk matmul needs them) ----
    x = [pers.tile([128, T], BF16, tag=f"x{i}", name=f"x{i}") for i in range(4)]
    for i in range(4):
        nc.sync.dma_start(x[i][:, ts(0, 512)],
                          ins[f'x{p}'].ap()[ts(i, 128), ts(0, 512)])
    wqk = [wp.tile([128, 1024], BF16, tag=f"{p}wqk{i}", name=f"wqk{i}") for i in range(4)]
    wv = [wp.tile([128, 512], BF16, tag=f"{p}wv{i}", name=f"wv{i}") for i in range(4)]
    wo = [wp.tile([128, 512], BF16, tag=f"{p}wo{i}", name=f"wo{i}") for i in range(4)]
    w1 = [wp.tile([128, ffd], BF16, tag=f"{p}w1_{i}", name=f"w1_{i}") for i in range(4)]
    w2 = [wp.tile([128, 512], BF16, tag=f"{p}w2_{i}", name=f"w2_{i}") for i in range(FJ)]
    for i in range(4):
        nc.sync.dma_start(wqk[i][:], ins[f'{p}_wqk'].ap()[ts(i, 128), :])
    for ch0 in range(1, NCH):
        for i in range(4):
            nc.sync.dma_start(x[i][:, ts(ch0, 512)],
                              ins[f'x{p}'].ap()[ts(i, 128), ts(ch0, 512)])
    for i in range(4):
        nc.sync.dma_start(wv[i][:], ins[f'{p}_wv'].ap()[ts(i, 128), :])
        nc.sync.dma_start(wo[i][:], ins[f'{p}_wo'].ap()[ts(i, 128), :])
        nc.sync.dma_start(w1[i][:], ins[f'{p}_w1'].ap()[ts(i, 128), :])
    for i in range(FJ):
        nc.sync.dma_start(w2[i][:], ins[f'{p}_w2'].ap()[ts(i, 128), :])

    if not trivial:
        bqk = wp.tile([128, 8], F32, tag="bqk", name="bqk")
        nc.sync.dma_start(bqk[:], ins[f'{p}_bqk'].ap().rearrange("(j q) one -> q (j one)", q=128))
        bo = wp.tile([128, 4], F32, tag="bo", name="bo")
        nc.sync.dma_start(bo[:], ins[f'{p}_bo'].ap().rearrange("(j q) one -> q (j one)", q=128))
        b1 = wp.tile([128, FJ], F32, tag="b1", name="b1")
        nc.sync.dma_start(b1[:], ins[f'{p}_b1'].ap().rearrange("(j q) one -> q (j one)", q=128))
        b2 = wp.tile([128, 4], F32, tag="b2", name="b2")
        nc.sync.dma_start(b2[:], ins[f'{p}_b2'].ap().rearrange("(j q) one -> q (j one)", q=128))
        ln1g = wp.tile([128, 4], F32, tag="ln1g", name="ln1g")
        nc.sync.dma_start(ln1g[:], ins[f'{p}_ln1g'].ap().rearrange("(j q) one -> q (j one)", q=128))
        ln1b = wp.tile([128, 4], F32, tag="ln1b", name="ln1b")
        nc.sync.dma_start(ln1b[:], ins[f'{p}_ln1b'].ap().rearrange("(j q) one -> q (j one)", q=128))
        ln2g = wp.tile([128, 4], F32, tag="ln2g", name="ln2g")
        nc.sync.dma_start(ln2g[:], ins[f'{p}_ln2g'].ap().rearrange("(j q) one -> q (j one)", q=128))
        ln2b = wp.tile([128, 4], F32, tag="ln2b", name="ln2b")
        nc.sync.dma_start(ln2b[:], ins[f'{p}_ln2b'].ap().rearrange("(j q) one -> q (j one)", q=128))
        # v-bias as a broadcast tile
        bvrow = wp.tile([1, 512], BF16, tag="bvrow", name="bvrow")
        nc.sync.dma_start(bvrow[:], ins[f'{p}_bv'].ap().rearrange("c one -> one c"))
        bv_bc = wp.tile([128, 512], BF16, tag="bvbc", name="bvbc")
        nc.gpsimd.partition_broadcast(bv_bc[:], bvrow[:])
    else:
        bqk = bo = b1 = b2 = ln1g = ln1b = ln2g = ln2b = bv_bc = None

    # ================= per 512-token chunk =====================
    # QKV for chunk ch+1 is emitted as 12 thunks interleaved into the
    # attention head loop of chunk ch, so the PE never starves while the
    # Activation engine cranks through the softmax exps.
    def emit_qkv_plan(ch):
        cols = ts(ch, 512)
        qksb = [strm.tile([128, 512], BF16, tag=f"qk{j}", bufs=2, name=f"qk{j}")
                for j in range(8)]
        # v tiles are (128, 8*65): per head 64 value cols + a ones column so
        # the AV matmul's stationary [v_h | 1] emits the softmax row-sum as
        # psum row 64 for free.
        vt = [strm.tile([128, 520], BF16, tag=f"v{ti}", bufs=2, name=f"v{ti}")
              for ti in range(4)]

        def qk_thunk(j):
            ps = psum.tile([128, 512], F32, tag="acc", bufs=5, name="qk_ps")
            for ci in range(4):
                nc.tensor.matmul(ps[:], wqk[ci][:, ts(j, 128)], x[ci][:, cols],
                                 start=(ci == 0), stop=(ci == 3))
            ev_copy(nc, asg['qk'][j], qksb[j][:], ps[:],
                    None if trivial else bqk[:, j:j + 1])

        def v_thunk(ti):
            ps = psum.tile([128, 512], F32, tag="acc", bufs=5, name="v_ps")
            for ci in range(4):
                nc.tensor.matmul(ps[:], x[ci][:, ch * 512 + ti * 128:ch * 512 + (ti + 1) * 128],
                                 wv[ci][:], start=(ci == 0), stop=(ci == 3))
            sb = vt[ti]
            dst = sb[:].rearrange("k (h e) -> k h e", e=65)[:, :, 0:64]
            src = ps[:].rearrange("k (h e) -> k h e", e=64)
            if trivial:
                ev_copy(nc, asg['v'][ti], dst, src)
            else:
                ev_res(nc, 'd', dst, src,
                       bv_bc[:].rearrange("k (h e) -> k h e", e=64))
            nc.vector.memset(sb[:].rearrange("k (h e) -> k h e", e=65)[:, :, 64:65], 1.0)

        thunks = [lambda j=j: qk_thunk(j) for j in range(8)]
        thunks += [lambda ti=ti: v_thunk(ti) for ti in range(4)]
        qh = [qksb[hb // 2][(hb % 2) * 64:(hb % 2) * 64 + 64, :] for hb in range(8)]
        kh = [qksb[4 + hb // 2][(hb % 2) * 64:(hb % 2) * 64 + 64, :] for hb in range(8)]
        return thunks, qh, kh, vt

    thunks, qh, kh, vt = emit_qkv_plan(0)
    for t in thunks:
        t()
    for ch in range(NCH):
        cols = ts(ch, 512)
        if ch + 1 < NCH:
            nthunks, nqh, nkh, nvt = emit_qkv_plan(ch + 1)
        else:
            nthunks = []
        ndone = [0]

        def drain_to(want, _nt=nthunks, _nd=ndone):
            while _nd[0] < min(want, len(_nt)):
                _nt[_nd[0]]()
                _nd[0] += 1

        def interleave(h, _d=drain_to):
            _d(((h + 1) * 11) // 8)

        # ---- attention -> o_fm tiles (4 x (128,512) bf16) ----
        o_fm = [strm.tile([128, 512], BF16, tag=f"o{j}", bufs=1, name=f"o{j}") for j in range(4)]
        if p in ('h', 'v'):
            _attn_hv(nc, psum, strm, tmp, bcp, qh, kh, vt, o_fm, ones_bf, asg,
                     interleave)
        else:
            _attn_s(nc, psum, strm, tmp, bcp, qh, kh, vt, o_fm, ones_bf, smask,
                    asg, interleave)
        # hold the last thunk back: it fills the PE bubble while the LN1
        # mean row is reduced/broadcast (emitted below)
        drain_to(11)
        if ch + 1 < NCH:
            qh, kh, vt = nqh, nkh, nvt

        # ---- out projection + residual (+ LN1 mean handling) ----
        for j in range(4):
            ps = psum.tile([128, 512], F32, tag="acc", bufs=5, name="y_ps")
            for ci in range(4):
                nc.tensor.matmul(ps[:], wo[ci][:, ts(j, 128)], o_fm[ci][:],
                                 start=(ci == 0), stop=(ci == 3))
            ev_res(nc, 'd', x[j][:, cols], ps[:], x[j][:, cols],
                   None if trivial else bo[:, j:j + 1])
        if trivial:
            # LN1 fast path: subtract the per-token mean only; the 1/std
            # scale commutes through the bias-free ReLU FF and cancels in
            # LN2 (positive per-token scale invariance).
            srow = psum.tile([33, 512], F32, tag="row", bufs=1, name="ln1_s1")
            for j in range(4):
                nc.tensor.matmul(srow[0:1, :], ones_bf[:], x[j][:, cols],
                                 start=(j == 0), stop=(j == 3))
            nm = tmp.tile([1, 512], BF16, tag="ln1_nm", bufs=1, name="ln1_nm")
            nc.vector.tensor_scalar(nm[:], srow[0:1, :], -1.0 / C, None, OP.mult)
            nm_bc = bcp.tile([128, 512], BF16, tag="ln1_nmbc", bufs=2, name="ln1_nmbc")
            nc.gpsimd.partition_broadcast(nm_bc[:], nm[:])
            drain_to(12)
            for j in range(4):
                ev_res(nc, asg['lnu'][j], x[j][:, cols], x[j][:, cols], nm_bc[:])
        else:
            _layernorm_full(nc, psum, tmp, bcp, x, cols, ln1g, ln1b, epst,
                            ones_bf, lambda j: (x[j][:, cols], None), tmp, asg)
            drain_to(12)

        # ---- ff1: relu(t @ w1 + b1) -> hb tiles ----
        hb = []
        for j in range(FJ):
            ps = psum.tile([128, 512], F32, tag="acc", bufs=5, name="f1_ps")
            for ci in range(4):
                nc.tensor.matmul(ps[:], w1[ci][:, ts(j, 128)], x[ci][:, cols],
                                 start=(ci == 0), stop=(ci == 3))
            sb = strm.tile([128, 512], BF16, tag=f"hb{j}", bufs=1, name=f"hb{j}")
            ev_relu(nc, asg['ff1'][j % len(asg['ff1'])], sb[:], ps[:],
                    None if trivial else b1[:, j:j + 1])
            hb.append(sb)
        # ---- ff2 + residual (overwrites x) ; LN2 -> dram ----
        for j in range(4):
            ps = psum.tile([128, 512], F32, tag="acc", bufs=5, name="f2_ps")
            for ci in range(FJ):
                nc.tensor.matmul(ps[:], w2[ci][:, ts(j, 128)], hb[ci][:],
                                 start=(ci == 0), stop=(ci == FJ - 1))
            ev_res(nc, 'd', x[j][:, cols], ps[:], x[j][:, cols],
                   None if trivial else b2[:, j:j + 1])

        def _dest(j, _ch=ch):
            zt = tmp.tile([128, 512], F32, tag="ln_out", bufs=2, name=f"lnout{j}")
            return (zt[:], lambda: nc.sync.dma_start(
                outs[f'y{p}'].ap()[ts(j, 128), ts(_ch, 512)], zt[:]))
        _layernorm_full(nc, psum, tmp, bcp, x, cols, ln2g, ln2b, epst,
                        ones_bf, _dest, tmp, asg)


def _layernorm_full(nc, psum, tmp, bcp, x, cols, g_t, b_t, epst, ones_bf,
                    dest_fn, tmpp, asg):
    """Full LayerNorm over C (partition dim) for one 512-token chunk.

    x: 4 bf16 tiles; operates on columns `cols`. A = rsqrt(var+eps) is
    computed as exp(-0.5*ln(var+eps)) to stay inside one act table.
    g_t/b_t None => trivial affine.
    """
    srow = psum.tile([33, 512], F32, tag="row", bufs=1, name="ln_s12")
    for j in range(4):
        nc.tensor.matmul(srow[0:1, :], ones_bf[:], x[j][:, cols],
                         start=(j == 0), stop=(j == 3))
    for j in range(4):
        tsq = tmpp.tile([128, 512], BF16, tag="ln_tsq", bufs=2, name="ln_tsq")
        ev_mult(nc, asg['tsq'][j], tsq[:], x[j][:, cols], x[j][:, cols])
        nc.tensor.matmul(srow[32:33, :], ones_bf[:], tsq[:],
                         start=(j == 0), stop=(j == 3))
    m = tmp.tile([1, 512], F32, tag="ln_m", bufs=1, name="ln_m")
    nc.vector.tensor_scalar(m[:], srow[0:1, :], 1.0 / C, None, OP.mult)
    mm = tmp.tile([1, 512], F32, tag="ln_mm", bufs=1, name="ln_mm")
    nc.vector.tensor_tensor(mm[:], m[:], m[:], OP.mult)
    var = tmp.tile([1, 512], F32, tag="ln_var", bufs=1, name="ln_var")
    nc.vector.scalar_tensor_tensor(var[:], srow[32:33, :], 1.0 / C, mm[:], OP.mult, OP.subtract)
    lnv = tmp.tile([1, 512], F32, tag="ln_lnv", bufs=1, name="ln_lnv")
    nc.scalar.activation(lnv[:], var[:], AF.Ln, bias=epst[:1, :])
    A = tmp.tile([1, 512], F32, tag="ln_A", bufs=1, name="ln_A")
    nc.scalar.activation(A[:], lnv[:], AF.Exp, scale=-0.5)
    Bv = tmp.tile([1, 512], F32, tag="ln_B", bufs=1, name="ln_B")
    nc.vector.scalar_tensor_tensor(Bv[:], m[:], -1.0, A[:], OP.mult, OP.mult)
    A_bc = bcp.tile([128, 512], F32, tag="ln_Abc", bufs=2, name="ln_Abc")
    nc.gpsimd.partition_broadcast(A_bc[:], A[:])
    B_bc = bcp.tile([128, 512], F32, tag="ln_Bbc", bufs=2, name="ln_Bbc")
    nc.gpsimd.partition_broadcast(B_bc[:], Bv[:])
    for j in range(4):
        u = tmp.tile([128, 512], F32, tag="ln_u", bufs=2, name="ln_u")
        ev_mult(nc, asg['z'][2 * j], u[:], x[j][:, cols], A_bc[:])
        dest, post = dest_fn(j)
        if g_t is None:
            ev_res(nc, asg['z'][2 * j + 1], dest, u[:], B_bc[:])
        else:
            z = tmp.tile([128, 512], F32, tag="ln_z", bufs=2, name="ln_z")
            nc.vector.tensor_tensor(z[:], u[:], B_bc[:], OP.add)
            nc.scalar.activation(dest, z[:], AF.Identity,
                                 bias=b_t[:, j:j + 1], scale=g_t[:, j:j + 1])
        if post is not None:
            post()


def _attn_hv(nc, psum, strm, tmp, bcp, qh, kh, vt, o_fm, ones_bf, asg, interleave, accb=5):
    """Attention for h/v blocks: one batch n per 512-chunk, L=512, 8 heads.

    The AV stationary is [v_h | 1] (65 wide), so o_ps row 64 holds the
    softmax row-sum -- no separate row-sum matmuls.
    """
    for h in range(8):
        off = (h % 2) * 64
        e_t = []
        for mt in range(4):
            sps = psum.tile([128, 512], F32, tag="acc", bufs=accb, name="s_ps")
            nc.tensor.matmul(sps[:], kh[h][:, ts(mt, 128)], qh[h][:],
                             start=True, stop=True)
            e = strm.tile([128, 512], BF16, tag="e", bufs=5, name="e")
            nc.scalar.activation(e[:], sps[:], AF.Exp, scale=0.125)
            e_t.append(e)
        o_ps = psum.tile([65, 512], F32, tag="ops", bufs=2, name="o_ps")
        for mt in range(4):
            nc.tensor.matmul(o_ps[:], vt[mt][:, h * 65:h * 65 + 65],
                             e_t[mt][:], start=(mt == 0), stop=(mt == 3))
        rinv = tmp.tile([1, 512], F32, tag="rinv", bufs=2, name="rinv")
        nc.vector.reciprocal(rinv[:], o_ps[64:65, :])
        R_bc = bcp.tile([64, 512], F32, tag="rbc", bufs=3, name="rbc")
        nc.gpsimd.partition_broadcast(R_bc[:], rinv[:])
        ev_mult(nc, asg['onorm'][h], o_fm[h // 2][off:off + 64, :],
                o_ps[0:64, :], R_bc[:])
        interleave(h)


def _attn_s(nc, psum, strm, tmp, bcp, qh, kh, vt, o_fm, ones_bf, smask, asg, interleave, accb=5):
    """Attention for the s block: 512-chunk = 32 batches of L=16; masked dense."""
    for h in range(8):
        off = (h % 2) * 64
        sps = psum.tile([128, 512], F32, tag="acc", bufs=accb, name="ss_ps")
        for g in range(4):
            nc.tensor.matmul(sps[:, ts(g, 128)], kh[h][:, ts(g, 128)],
                             qh[h][:, ts(g, 128)], start=True, stop=True)
        e = strm.tile([128, 512], BF16, tag="es", bufs=3, name="es")
        nc.scalar.activation(e[:], sps[:], AF.Exp, scale=0.125)
        em = strm.tile([128, 512], BF16, tag="em", bufs=3, name="em")
        ev_mult(nc, asg['em'][h], em[:], e[:], smask[:])
        o_ps = psum.tile([65, 512], F32, tag="ops", bufs=2, name="os_ps")
        for g in range(4):
            nc.tensor.matmul(o_ps[:, ts(g, 128)], vt[g][:, h * 65:h * 65 + 65],
                             em[:, ts(g, 128)], start=True, stop=True)
        rinv = tmp.tile([1, 512], F32, tag="rinv", bufs=2, name="rinvs")
        nc.vector.reciprocal(rinv[:], o_ps[64:65, :])
        R_bc = bcp.tile([64, 512], F32, tag="rbc", bufs=3, name="rbcs")
        nc.gpsimd.partition_broadcast(R_bc[:], rinv[:])
        ev_mult(nc, asg['onorm'][h], o_fm[h // 2][off:off + 64, :],
                o_ps[0:64, :], R_bc[:])
        interleave(h)


# ---------------------------------------------------------------------------
# Launch 2 device program
# ---------------------------------------------------------------------------

def build_launch2():
    nc = bacc.Bacc("TRN2", target_bir_lowering=False, debug=False, num_devices=N_CORES)
    NPIX = 1024
    ins = {}
    for bb in range(2):
        ins[f'hc{bb}'] = nc.dram_tensor(f'hc{bb}', [C, NPIX], F32R, kind="ExternalInput")
        ins[f'vc{bb}'] = nc.dram_tensor(f'vc{bb}', [C, NPIX], F32R, kind="ExternalInput")
        ins[f'q4_{bb}'] = nc.dram_tensor(f'q4_{bb}', [C, NPIX], F32R, kind="ExternalInput")
    ins['c1w'] = nc.dram_tensor('c1w', [2 * C, C], F32R, kind="ExternalInput")
    ins['c2w'] = nc.dram_tensor('c2w', [C, C], F32R, kind="ExternalInput")
    ins['c1b'] = nc.dram_tensor('c1b', [C, 1], F32, kind="ExternalInput")
    ins['c2b'] = nc.dram_tensor('c2b', [C, 1], F32, kind="ExternalInput")
    ins['onesv'] = nc.dram_tensor('onesv', [128, 2], F32R, kind="ExternalInput")
    ins['ident'] = nc.dram_tensor('ident', [128, 128], F32R, kind="ExternalInput")
    outs = {}
    for bb in range(2):
        outs[f'out{bb}'] = nc.dram_tensor(f'out{bb}', [C, NPIX], F32, kind="ExternalOutput")

    with tile.TileContext(nc) as tc:
        from contextlib import ExitStack
        with ExitStack() as ctx:
            ctx.enter_context(nc.allow_low_precision(reason="fp32r matmul input pipeline"))
            const = ctx.enter_context(tc.tile_pool(name="const", bufs=1))
            ones128 = const.tile([128, 2], F32R, tag="ones128", name="ones128")
            nc.sync.dma_start(ones128[:], ins['onesv'].ap()[:, :])
            onesrow = const.tile([1, 128], F32R, tag="onesrow", name="onesrow")
            nc.sync.dma_start(onesrow[:], ins['onesv'].ap()[:, 0:1].rearrange("c one -> one c"))
            nshift = const.tile([128, 1], F32, tag="nshift", name="nshift")
            nc.vector.memset(nshift[:], -XSHIFT)
            ident = const.tile([128, 128], F32R, tag="ident", name="ident")
            nc.sync.dma_start(ident[:], ins['ident'].ap()[:, :])
            wp = ctx.enter_context(tc.tile_pool(name="w", bufs=1))
            c1w = [wp.tile([128, 512], F32R, tag=f"c1w{i}", name=f"c1w{i}") for i in range(8)]
            for i in range(8):
                nc.sync.dma_start(c1w[i][:], ins['c1w'].ap()[ts(i, 128), :])
            c2w = [wp.tile([128, 512], F32R, tag=f"c2w{i}", name=f"c2w{i}") for i in range(4)]
            for i in range(4):
                nc.sync.dma_start(c2w[i][:], ins['c2w'].ap()[ts(i, 128), :])
            c1b = wp.tile([128, 4], F32, tag="c1b", name="c1b")
            nc.sync.dma_start(c1b[:], ins['c1b'].ap().rearrange("(j q) one -> q (j one)", q=128))
            c2b = wp.tile([128, 4], F32, tag="c2b", name="c2b")
            nc.sync.dma_start(c2b[:], ins['c2b'].ap().rearrange("(j q) one -> q (j one)", q=128))
            c1brow = wp.tile([1, 512], F32R, tag="c1brow", name="c1brow")
            nc.sync.dma_start(c1brow[:], ins['c1b'].ap().rearrange("c one -> one c").bitcast(F32R))
            psum0 = ctx.enter_context(tc.tile_pool(name="ps0", bufs=1, space="PSUM"))
            c1b_bc = wp.tile([128, 512], F32, tag="c1bbc", name="c1bbc")
            nc.gpsimd.partition_broadcast(c1b_bc[:], c1brow[:].bitcast(F32))

            dramp = ctx.enter_context(tc.tile_pool(name="dram", bufs=2, space="DRAM"))

            pers2 = ctx.enter_context(tc.tile_pool(name="b_pers", bufs=1))
            psum2 = ctx.enter_context(tc.tile_pool(name="b_ps", bufs=1, space="PSUM"))
            strm2 = ctx.enter_context(tc.tile_pool(name="b_strm", bufs=1))
            for bb in range(2):
                _launch2_b(tc, ctx, nc, bb, ins, outs, c1w, c2w, c1b, c2b, c1b_bc,
                           ones128, nshift, dramp, pers2, psum2, strm2, ident)
    nc.compile()
    return nc


def _launch2_b(tc, octx, nc, bb, ins, outs, c1w, c2w, c1b, c2b, c1b_bc,
               ones128, nshift, dramp, pers, psum, strm, ident):
    if True:
        hc = [pers.tile([128, 1024], F32R, tag=f"hc{i}", name=f"hc{i}") for i in range(4)]
        vc = [pers.tile([128, 1024], F32R, tag=f"vc{i}", name=f"vc{i}") for i in range(4)]
        q4 = [pers.tile([128, 1024], F32R, tag=f"q4{i}", name=f"q4{i}") for i in range(4)]
        for i in range(4):
            nc.sync.dma_start(hc[i][:], ins[f'hc{bb}'].ap()[ts(i, 128), :])
            nc.sync.dma_start(vc[i][:], ins[f'vc{bb}'].ap()[ts(i, 128), :])
            nc.sync.dma_start(q4[i][:], ins[f'q4_{bb}'].ap()[ts(i, 128), :])
        fused = hc + vc

        # conv1 feature-major: gm_fm (512, 1024)
        gm_fm = [pers.tile([128, 1024], F32R, tag=f"gmf{j}", name=f"gmf{j}") for j in range(4)]
        for pj in range(2):
            for j in range(4):
                ps = psum.tile([128, 512], F32, tag="acc", bufs=5, name="g1_ps")
                for ci in range(8):
                    nc.tensor.matmul(ps[:], r32(c1w[ci][:, ts(j, 128)]),
                                     r32(fused[ci][:, ts(pj, 512)]),
                                     start=(ci == 0), stop=(ci == 7))
                nc.scalar.activation(gm_fm[j][:, ts(pj, 512)], ps[:], AF.Identity,
                                     bias=c1b[:, j:j + 1])
        # conv1 token-major via PE transpose of gm_fm (bias already applied)
        gm_tok = [pers.tile([128, 512], F32R, tag=f"gmt{t}", name=f"gmt{t}") for t in range(8)]
        for t in range(8):
            ps = psum.tile([128, 512], F32, tag="acc", bufs=5, name="g2_ps")
            for ci in range(4):
                nc.tensor.transpose(r32(ps[:, ts(ci, 128)]), r32(gm_fm[ci][:, ts(t, 128)]),
                                    ident[:])
            if t % 2 == 0:
                nc.vector.tensor_copy(gm_tok[t][:], ps[:])
            else:
                nc.scalar.activation(gm_tok[t][:], ps[:], AF.Identity)

        # scores + exp: e (1024 m, 1024 t)
        e_t = [pers.tile([128, 1024], F32R, tag=f"e{mt}", name=f"e{mt}") for mt in range(8)]
        for mt in range(8):
            for tj in range(2):
                ps = psum.tile([128, 512], F32, tag="acc", bufs=5, name="sc_ps")
                for ci in range(4):
                    nc.tensor.matmul(ps[:], r32(gm_fm[ci][:, ts(mt, 128)]),
                                     r32(q4[ci][:, ts(tj, 512)]),
                                     start=(ci == 0), stop=(ci == 3))
                nc.scalar.activation(e_t[mt][:, ts(tj, 512)], ps[:], AF.Exp,
                                     bias=nshift[:, :])
        # att (token-major) + row sums; normalize via per-partition recip
        att_dram = dramp.tile([1024, 512], F32, tag="attd", name="attd")
        for tt in range(8):
            aps = psum.tile([128, 512], F32, tag="acc", bufs=5, name="at_ps")
            rps = psum.tile([128, 2], F32, tag="row", bufs=1, name="r_ps")
            for mt in range(8):
                nc.tensor.matmul(aps[:], r32(e_t[mt][:, ts(tt, 128)]), r32(gm_tok[mt][:]),
                                 start=(mt == 0), stop=(mt == 7))
            for mt in range(8):
                nc.tensor.matmul(rps[:], r32(e_t[mt][:, ts(tt, 128)]), r32(ones128[:]),
                                 start=(mt == 0), stop=(mt == 7))
            rinv = strm.tile([128, 1], F32, tag="rinv", bufs=1, name="rinv")
            nc.vector.reciprocal(rinv[:], rps[:, 0:1])
            att = strm.tile([128, 512], F32, tag="att", bufs=3, name="att")
            nc.vector.tensor_scalar(att[:], aps[:], rinv[:], None, OP.mult)
            nc.sync.dma_start(att_dram[ts(tt, 128), :], att[:])

        # scrambled view: S_i = flat(att_i) as (512, 256); z = gm + S; conv2
        z_t = [pers.tile([128, 1024], F32R, tag=f"z{j}", name=f"z{j}") for j in range(4)]
        att_flat = att_dram[:].rearrange("n c -> (n c)")
        for i in range(4):
            y0 = (i // 2) * 16
            x0 = (i % 2) * 16
            for j in range(4):
                S = strm.tile([128, 256], F32, tag="S", bufs=3, name="S")
                src = att_flat[i * 131072 + j * 32768: i * 131072 + (j + 1) * 32768]
                nc.sync.dma_start(S[:], src.rearrange("(q f) -> q f", q=128))
                gm_view = gm_fm[j][:].rearrange("q (h w) -> q h w", h=32)[
                    :, y0:y0 + 16, x0:x0 + 16]
                nc.vector.tensor_tensor(z_t[j][:, i * 256:(i + 1) * 256].rearrange("q (y x) -> q y x", y=16),
                                        S[:].rearrange("q (y x) -> q y x", y=16),
                                        gm_view, OP.add)
        # conv2: out = c2w @ z + c2b
        for pj in range(2):
            for j in range(4):
                ps = psum.tile([128, 512], F32, tag="acc", bufs=5, name="o_ps")
                for ci in range(4):
                    nc.tensor.matmul(ps[:], r32(c2w[ci][:, ts(j, 128)]),
                                     r32(z_t[ci][:, ts(pj, 512)]),
                                     start=(ci == 0), stop=(ci == 3))
                ot = strm.tile([128, 512], F32, tag="ot", bufs=3, name="ot")
                nc.scalar.activation(ot[:], ps[:], AF.Identity, bias=c2b[:, j:j + 1])
                nc.sync.dma_start(outs[f'out{bb}'].ap()[ts(j, 128), ts(pj, 512)], ot[:])


# ---------------------------------------------------------------------------
# Host-side sharding / resharding
# ---------------------------------------------------------------------------

def make_smask():
    m = np.zeros((128, 128), np.float32)
    for n in range(8):
        m[n * 16:(n + 1) * 16, n * 16:(n + 1) * 16] = 1.0
    return np.tile(m, (1, 4)).copy()


def _is_trivial(inputs):
    for p in ('h', 'v', 's'):
        for k, want in ((f'{p}_qkv_b', 0.0), (f'{p}_out_b', 0.0),
                        (f'{p}_ff1_b', 0.0), (f'{p}_ff2_b', 0.0),
                        (f'{p}_ln1_g', 1.0), (f'{p}_ln1_b', 0.0),
                        (f'{p}_ln2_g', 1.0), (f'{p}_ln2_b', 0.0)):
            if not np.all(np.asarray(inputs[k]) == want):
                return False
    return True


def block_weights(inputs, p, trivial):
    wqkv = np.asarray(inputs[f'{p}_qkv_w'], np.float32)
    wo = np.asarray(inputs[f'{p}_out_w'], np.float32)
    if trivial:
        # Fold LN1's mean subtraction of the attention term into Wo:
        # (Wo - colmean(Wo)) o == Wo o - mean_f(Wo o) per token.
        wo = wo - wo.mean(axis=0, keepdims=True)
    d = {
        f'{p}_wqk': np.ascontiguousarray(wqkv[:1024].T).astype(BF),
        f'{p}_wv': np.ascontiguousarray(wqkv[1024:1536].T).astype(BF),
        f'{p}_wo': np.ascontiguousarray(wo.T).astype(BF),
        f'{p}_w1': np.ascontiguousarray(np.asarray(inputs[f'{p}_ff1_w'], np.float32).T).astype(BF),
        f'{p}_w2': np.ascontiguousarray(np.asarray(inputs[f'{p}_ff2_w'], np.float32).T).astype(BF),
    }
    if not trivial:
        d[f'{p}_bqk'] = np.asarray(inputs[f'{p}_qkv_b'][:1024], np.float32).reshape(-1, 1)
        d[f'{p}_bv'] = np.asarray(inputs[f'{p}_qkv_b'][1024:1536], np.float32).reshape(-1, 1).astype(BF)
        d[f'{p}_bo'] = np.asarray(inputs[f'{p}_out_b'], np.float32).reshape(-1, 1)
        d[f'{p}_b1'] = np.asarray(inputs[f'{p}_ff1_b'], np.float32).reshape(-1, 1)
        d[f'{p}_b2'] = np.asarray(inputs[f'{p}_ff2_b'], np.float32).reshape(-1, 1)
        d[f'{p}_ln1g'] = np.asarray(inputs[f'{p}_ln1_g'], np.float32).reshape(-1, 1)
        d[f'{p}_ln1b'] = np.asarray(inputs[f'{p}_ln1_b'], np.float32).reshape(-1, 1)
        d[f'{p}_ln2g'] = np.asarray(inputs[f'{p}_ln2_g'], np.float32).reshape(-1, 1)
        d[f'{p}_ln2b'] = np.asarray(inputs[f'{p}_ln2_b'], np.float32).reshape(-1, 1)
    return d


def make_bq(fm):
    b, c, h, w = fm.shape
    y = fm.reshape(b, 32, 16, 2, 16, w)
    y = np.transpose(y, (0, 1, 3, 5, 2, 4))
    blocks = y.reshape(b, c, 2, 2, 16, 16)
    blk = np.transpose(blocks, (0, 2, 3, 1, 4, 5)).reshape(b, 4, c, 256)
    bq = np.transpose(blk, (0, 1, 3, 2)).reshape(b, 4 * 256, c)
    return np.ascontiguousarray(bq)


def shard_launch1(inputs, trivial):
    fm = np.asarray(inputs['feature_map'], np.float32)
    b, c, h, w = fm.shape
    xh_full = np.transpose(fm, (0, 2, 3, 1)).reshape(b * h, w, c)   # (L, W, C)
    xv_full = np.transpose(fm, (0, 3, 2, 1)).reshape(b * w, h, c)   # (L, H, C)
    bq = make_bq(fm)                                                # (B, 1024, C)
    weights = {}
    for p in ('h', 'v', 's'):
        weights.update(block_weights(inputs, p, trivial))
    weights['smask'] = make_smask().astype(BF)
    if not trivial:
        weights['onesv'] = np.ones((128, 1), BF)
    in_maps = []
    for core in range(N_CORES):
        m = dict(weights)
        xh = xh_full[:, core * 4:(core + 1) * 4, :]          # (512, 4, C)
        xv = xv_full[:, core * 4:(core + 1) * 4, :]
        xs = bq[:, core * 128:(core + 1) * 128, :]           # (16, 128, C)
        for key, arr in (('xh', xh), ('xv', xv), ('xs', xs)):
            xf = np.ascontiguousarray(
                np.transpose(arr, (2, 1, 0)).reshape(C, T)).astype(np.float32)
            m[f'x{key[1:]}'] = xf.astype(BF)
            if trivial:
                m[f'negmx{key[1:]}'] = (-xf.mean(axis=0, keepdims=True)).astype(BF)
        in_maps.append(m)
    return in_maps


def _host_ln2(y):
    """Exact LN2 (trivial affine) over the feature axis of a (C, T) array."""
    m = y.mean(axis=0, keepdims=True)
    v = y.var(axis=0, keepdims=True)
    return (y - m) / np.sqrt(v + EPS)


def reshard_launch2(results1, inputs, trivial=False):
    """results1: list per core of {'yh','yv','ys'} feature-major (C, 2048).

    In the trivial fast path the device emits pre-LN2 activations (bf16);
    LN2 is applied here exactly (launch2's inputs are linear in the LN2
    output, so this is a pure refactor of where the normalize happens).
    """
    Hc = np.zeros((B, C, HW, HW), np.float32)
    Vc = np.zeros((B, C, HW, HW), np.float32)
    bq2 = np.zeros((B, 1024, C), np.float32)
    for core in range(N_CORES):
        yh = np.asarray(results1[core]['yh'], dtype=np.float32)
        yv = np.asarray(results1[core]['yv'], dtype=np.float32)
        ys = np.asarray(results1[core]['ys'], dtype=np.float32)
        if trivial:
            yh = _host_ln2(yh)
            yv = _host_ln2(yv)
            ys = _host_ln2(ys)
        yh = yh.reshape(C, 4, B, HW)
        Hc[:, :, :, core * 4:(core + 1) * 4] = np.transpose(yh, (2, 0, 3, 1))
        yv = yv.reshape(C, 4, B, HW)
        Vc[:, :, core * 4:(core + 1) * 4, :] = np.transpose(yv, (2, 0, 1, 3))
        ys = ys.reshape(C, 128, B)
        bq2[:, core * 128:(core + 1) * 128, :] = np.transpose(ys, (2, 1, 0))
    c1w = np.ascontiguousarray(np.asarray(inputs['conv1_w'], np.float32).T)  # (1024, 512)
    c2w = np.ascontiguousarray(np.asarray(inputs['conv2_w'], np.float32).T)  # (512, 512)
    c1b = np.asarray(inputs['conv1_b'], np.float32).reshape(-1, 1)
    c2b = np.asarray(inputs['conv2_b'], np.float32).reshape(-1, 1)
    in_maps = []
    for core in range(N_CORES):
        m = {'c1w': c1w, 'c2w': c2w, 'c1b': c1b, 'c2b': c2b,
             'onesv': np.ones((128, 2), np.float32),
             'ident': np.eye(128, dtype=np.float32)}
        for bb in range(2):
            b_idx = core * 2 + bb
            m[f'hc{bb}'] = np.ascontiguousarray(Hc[b_idx].reshape(C, 1024))
            m[f'vc{bb}'] = np.ascontiguousarray(Vc[b_idx].reshape(C, 1024))
            m[f'q4_{bb}'] = np.ascontiguousarray(bq2[b_idx].T)
        in_maps.append(m)
    return in_maps, (Hc, Vc, bq2)


def unshard_output(results2):
    out = np.zeros((B, C, HW, HW), np.float32)
    for core in range(N_CORES):
        for bb in range(2):
            b_idx = core * 2 + bb
            ob = results2[core][f'out{bb}']                   # (C, 1024) beta-order
            ob = ob.reshape(C, 2, 2, 16, 16)
            out[b_idx] = np.transpose(ob, (0, 1, 3, 2, 4)).reshape(C, HW, HW)
    return out


# ---------------------------------------------------------------------------
# Entry point
# ---------------------------------------------------------------------------

_CACHE = {}


def _programs(trivial=True):
    key = ('nc1', trivial)
    if key not in _CACHE:
        _CACHE[key] = build_launch1(trivial)
    if 'nc2' not in _CACHE:
        _CACHE['nc2'] = build_launch2()
    return _CACHE[key], _CACHE['nc2']


def kernel(**inputs) -> np.ndarray:
    from concourse import bass_utils
    trivial = _is_trivial(inputs)
    nc1, nc2 = _programs(trivial)
    in_maps1 = shard_launch1(inputs, trivial)
    r1 = bass_utils.run_bass_kernel_spmd(nc1, in_maps1, core_ids=list(range(N_CORES)))
    in_maps2, _ = reshard_launch2(r1.results, inputs, trivial)
    r2 = bass_utils.run_bass_kernel_spmd(nc2, in_maps2, core_ids=list(range(N_CORES)))
    return unshard_output(r2.results)

